# revision 6
# baseline (speedup 1.0000x reference)
"""AgriMatcher Trainium2 kernel: point-matching network + weighted-DLT homography.

Data-parallel over batch B=64 across 8 NeuronCores (8 images/core). The device
runs the network (fc-compression + LayerNorm + gelu, PointNet encoder, weight
head) and accumulates the per-image 9x9 weighted Gram matrix
M = sum_n w_n q_n q_n^T over Hartley-normalized point monomials q (host-built).
Host assembles AtWA/AtWb from M, solves 8x8, composes the 3x3 homographies.

Perf structure:
- fc1 and all five 128-wide layers run as fp8(E4M3) DoubleRow matmuls
  (2 fp8 rows/PE-cell = 2x-4x tensor throughput). Weights carry per-row pow2
  scales, undone by each evacuation's per-partition scale; layer biases ride a
  static ones-row in the DoubleRow zero-plane.
- Host precomputes |fA-fB| and fA*fB (fp8, pow2-scaled), the DLT q monomials,
  and a 3-way fp8 hi/mid/lo split of the positions for the encoder input.
- LayerNorm via PE transposes; variance by fused square+accumulate
  (scalar_tensor_tensor) on Vector; rstd (fast-invsqrt + Newton) on Vector;
  per-chunk rstd apply on GpSimd.
- Evacuations (PSUM->SBUF w/ relu+scale) balanced across Scalar and Vector.
"""

import numpy as np
import ml_dtypes

import concourse.bass as bass
import concourse.mybir as mybir
import concourse.tile as tile
from concourse import bacc, bass_utils
from concourse.masks import make_identity

F32 = mybir.dt.float32
BF16 = mybir.dt.bfloat16
FP8 = mybir.dt.float8e4
I32 = mybir.dt.int32
AF = mybir.ActivationFunctionType
OP = mybir.AluOpType
AX = mybir.AxisListType
PM = mybir.MatmulPerfMode

B, N, C = 64, 4096, 128
HID, COMP = 128, 32
NCORES = 8
BL = B // NCORES          # images per core
TILE = 1024               # points per tile
NT = N // TILE            # tiles per image (4)
NCH = TILE // 128         # 128-pt chunks per tile (8)
NTC = BL * NT             # tiles per core (32)
NC32 = N // 128           # 128-pt chunks per image (32)
EPS = 1e-5
REG = 1e-4
MAGIC = 0x5F3759DF

BF = ml_dtypes.bfloat16
E4 = ml_dtypes.float8_e4m3


def build():
    nc = bacc.Bacc("TRN2", target_bir_lowering=False, debug=False,
                   num_devices=NCORES)

    dm = nc.dram_tensor("dm", [BL, 128, 2, N], FP8, kind="ExternalInput").ap()
    posq = nc.dram_tensor("posq", [BL, 12, N], FP8, kind="ExternalInput").ap()
    qh = nc.dram_tensor("qh", [128, BL, 9, NC32], F32,
                        kind="ExternalInput").ap()
    zpl = nc.dram_tensor("zpl", [128, TILE], FP8, kind="ExternalInput").ap()
    # params
    w1q = nc.dram_tensor("w1q", [128, 2, 64], FP8, kind="ExternalInput").ap()
    sfc = nc.dram_tensor("sfc", [64, 1], F32, kind="ExternalInput").ap()
    b1c = nc.dram_tensor("b1c", [64, 1], F32, kind="ExternalInput").ap()
    gcol = nc.dram_tensor("gcol", [64, 1], F32, kind="ExternalInput").ap()
    bln = nc.dram_tensor("bln", [64, 1], F32, kind="ExternalInput").ap()
    we0 = nc.dram_tensor("we0", [76, 2, 128], FP8, kind="ExternalInput").ap()
    s0c = nc.dram_tensor("s0c", [128, 1], F32, kind="ExternalInput").ap()
    we1 = nc.dram_tensor("we1", [128, 2, 128], FP8, kind="ExternalInput").ap()
    s1c = nc.dram_tensor("s1c", [128, 1], F32, kind="ExternalInput").ap()
    we2 = nc.dram_tensor("we2", [128, 2, 128], FP8, kind="ExternalInput").ap()
    s2c = nc.dram_tensor("s2c", [128, 1], F32, kind="ExternalInput").ap()
    w0a = nc.dram_tensor("w0a", [128, 2, 128], FP8, kind="ExternalInput").ap()
    s3c = nc.dram_tensor("s3c", [128, 1], F32, kind="ExternalInput").ap()
    w0b = nc.dram_tensor("w0b", [128, 128], BF16, kind="ExternalInput").ap()
    bh0c = nc.dram_tensor("bh0c", [128, 1], F32, kind="ExternalInput").ap()
    wh1 = nc.dram_tensor("wh1", [128, 2, 64], FP8, kind="ExternalInput").ap()
    s4c = nc.dram_tensor("s4c", [64, 1], F32, kind="ExternalInput").ap()
    w2col = nc.dram_tensor("w2col", [64, 1], BF16, kind="ExternalInput").ap()
    tb2 = nc.dram_tensor("tb2", [128, 1], F32, kind="ExternalInput").ap()

    out = nc.dram_tensor("out", [BL, 9, 9], F32, kind="ExternalOutput").ap()

    with tile.TileContext(nc) as tc:
        with (
            tc.tile_pool(name="const", bufs=1) as cp,
            tc.tile_pool(name="persist", bufs=1) as pp,
            tc.tile_pool(name="work", bufs=3) as wp,
            tc.tile_pool(name="feat", bufs=4) as fp,
            tc.tile_pool(name="ps", bufs=2, space="PSUM") as ps,
            tc.tile_pool(name="psb", bufs=2, space="PSUM") as psb,
        ):
            ident = cp.tile([128, 128], BF16)
            make_identity(nc, ident)

            def cload(ap_in, shape, dtype):
                t = cp.tile(shape, dtype, tag=ap_in.tensor.name)
                nc.sync.dma_start(out=t, in_=ap_in)
                return t

            w1q_t = cload(w1q, [128, 2, 64], FP8)
            sfc_t = cload(sfc, [64, 1], F32)
            b1c_t = cload(b1c, [64, 1], F32)
            gcol_t = cload(gcol, [64, 1], F32)
            bln_t = cload(bln, [64, 1], F32)
            we0_t = cload(we0, [76, 2, 128], FP8)
            s0c_t = cload(s0c, [128, 1], F32)
            we1_t = cload(we1, [128, 2, 128], FP8)
            s1c_t = cload(s1c, [128, 1], F32)
            we2_t = cload(we2, [128, 2, 128], FP8)
            s2c_t = cload(s2c, [128, 1], F32)
            w0a_t = cload(w0a, [128, 2, 128], FP8)
            s3c_t = cload(s3c, [128, 1], F32)
            w0b_t = cload(w0b, [128, 128], BF16)
            bh0c_t = cload(bh0c, [128, 1], F32)
            wh1_t = cload(wh1, [128, 2, 64], FP8)
            s4c_t = cload(s4c, [64, 1], F32)
            w2col_t = cload(w2col, [64, 1], BF16)
            tb2_t = cload(tb2, [128, 1], F32)

            qh_sb = pp.tile([128, BL, 9, NC32], F32)
            nc.sync.dma_start(out=qh_sb, in_=qh)

            # persistent state
            hc_all = pp.tile([128, NTC, NCH, 64], BF16)
            s2_all = pp.tile([128, NTC, NCH], F32)
            rstd_all = pp.tile([128, NTC * NCH], F32)
            vp_all = pp.tile([128, NTC * NCH], F32)
            u_all = pp.tile([128, NTC * NCH], F32)
            w_all = pp.tile([128, BL, NC32], F32)
            gparts = pp.tile([128, BL, NT], F32)

            # fp8 activation tiles: [*, 2, TILE], plane 1 = zero pad with
            # ones at partitions 0/1 (DoubleRow bias rows). DMA'd once.
            hg_b = [pp.tile([76, 2, TILE], FP8, tag=f"hg{i}", name=f"hg{i}")
                    for i in range(2)]
            x1_b = [pp.tile([128, 2, TILE], FP8, tag=f"x1{i}", name=f"x1{i}")
                    for i in range(2)]
            x2_b = [pp.tile([128, 2, TILE], FP8, tag=f"x2{i}", name=f"x2{i}")
                    for i in range(2)]
            y0_b = [pp.tile([128, 2, TILE], FP8, tag=f"y0{i}", name=f"y0{i}")
                    for i in range(2)]
            # local: plane 0..3 = data tiles, plane 4 = zero pad
            loc_b = [pp.tile([128, NT + 1, TILE], FP8, tag=f"lc{i}",
                             name=f"lc{i}") for i in range(2)]
            y1_b = [pp.tile([64, TILE], BF16, tag=f"y1{i}", name=f"y1{i}")
                    for i in range(2)]

            for t in hg_b:
                nc.sync.dma_start(out=t[0:76, 1, :], in_=zpl[0:76, :])
            for t in x1_b + x2_b + y0_b:
                nc.sync.dma_start(out=t[:, 1, :], in_=zpl)
            for t in loc_b:
                nc.sync.dma_start(out=t[:, NT, :], in_=zpl)

            def dr_rhs(t, plane, zplane, sl):
                base = t[:, plane, sl]
                return bass.AP(tensor=base.tensor, offset=base.offset,
                               ap=[base.ap[0],
                                   [(zplane - plane) * TILE, 2],
                                   base.ap[-1]])

            # ---------------- phase 1: fc1 + LN stats ----------------
            def p1_load(st):
                img, ti = st["img"], st["ti"]
                p0 = ti * TILE
                dm_t = fp.tile([128, 2, TILE], FP8, tag="dm")
                nc.sync.dma_start(out=dm_t, in_=dm[img, :, :, p0:p0 + TILE])
                st["dm"] = dm_t

            def p1_fc1(st):
                h_ps = ps.tile([64, TILE], F32, tag="big")
                dm_t = st["dm"]
                for half in range(TILE // 512):
                    sl = slice(half * 512, half * 512 + 512)
                    nc.tensor.matmul(h_ps[:, sl], w1q_t, dm_t[:, :, sl],
                                     start=True, stop=True,
                                     perf_mode=PM.DoubleRow)
                st["h_ps"] = h_ps

            def p1_evac(st):
                h_sb = wp.tile([64, TILE], BF16, tag="h_sb")
                nc.scalar.activation(h_sb, st["h_ps"], AF.Identity,
                                     bias=b1c_t, scale=sfc_t)
                st["h_sb"] = h_sb

            def p1_fwdT(st):
                hp_ps = psb.tile([128, NCH, 64], BF16, tag="tp")
                h_sb = st["h_sb"]
                for j in range(NCH):
                    nc.tensor.transpose(hp_ps[:, j, :],
                                        h_sb[:, j * 128:(j + 1) * 128],
                                        ident[:64, :64])
                st["hp_ps"] = hp_ps

            def p1_sq(st):
                t = st["t"]
                hp = st["hp_ps"]
                nc.vector.tensor_copy(
                    hc_all[:, t].rearrange("p a b -> p (a b)"),
                    hp.rearrange("p a b -> p (a b)"))
                sqd = wp.tile([128, NCH, 64], BF16, tag="sqd")
                hc = hc_all[:, t]
                for j in range(NCH):
                    nc.vector.scalar_tensor_tensor(
                        out=sqd[:, j], in0=hc[:, j], scalar=0.0,
                        in1=hc[:, j], op0=OP.bypass, op1=OP.mult,
                        accum_out=s2_all[:, t, j:j + 1])

            P1_STAGES = [p1_fc1, p1_evac, p1_fwdT, p1_sq]

            def newton_all():
                s2f = s2_all.rearrange("p a b -> p (a b)")
                vp, yv, u_t = vp_all, rstd_all, u_all
                nc.vector.tensor_scalar(vp, s2f, 1.0 / 64.0, EPS,
                                        op0=OP.mult, op1=OP.add)
                nc.vector.tensor_scalar(yv.bitcast(I32), vp.bitcast(I32), 1,
                                        None, op0=OP.arith_shift_right)
                nc.vector.tensor_scalar(yv.bitcast(I32), yv.bitcast(I32),
                                        0xFFFFFFFF, None, op0=OP.bitwise_xor)
                nc.vector.tensor_scalar(yv.bitcast(I32), yv.bitcast(I32),
                                        MAGIC + 1, None, op0=OP.add)
                for _ in range(3):
                    nc.vector.tensor_mul(u_t, yv, yv)
                    nc.vector.tensor_mul(u_t, u_t, vp)
                    nc.vector.tensor_scalar(u_t, u_t, -0.5, 1.5,
                                            op0=OP.mult, op1=OP.add)
                    nc.vector.tensor_mul(yv, yv, u_t)

            # ---------------- phase 2 stages ----------------
            def e_apply(st):
                t = st["t"]
                hcn = wp.tile([128, NCH, 64], BF16, tag="hcn")
                for j in range(NCH):
                    nc.gpsimd.tensor_scalar(
                        hcn[:, j], hc_all[:, t, j],
                        rstd_all[:, t * NCH + j:t * NCH + j + 1], None,
                        op0=OP.mult)
                st["hcn"] = hcn
                # prefetch positions into the hg buffer
                img, ti = st["img"], st["ti"]
                p0 = ti * TILE
                hg_t = st["hg"]
                nc.sync.dma_start(out=hg_t[64:76, 0, :],
                                  in_=posq[img, :, p0:p0 + TILE])

            def e_bwdT(st):
                ycm = psb.tile([64, TILE], BF16, tag="tp")
                hcn = st["hcn"]
                for j in range(NCH):
                    nc.tensor.transpose(ycm[:, j * 128:(j + 1) * 128],
                                        hcn[:, j], ident)
                st["ycm"] = ycm

            def e_gelu(st):
                hg_t = st["hg"]
                nc.scalar.activation(hg_t[0:64, 0, :], st["ycm"], AF.Gelu,
                                     bias=bln_t, scale=gcol_t)

            def e_enc0(st):
                e0 = ps.tile([128, TILE], F32, tag="big")
                hg_t = st["hg"]
                for half in range(TILE // 512):
                    sl = slice(half * 512, half * 512 + 512)
                    nc.tensor.matmul(e0[:, sl], we0_t,
                                     dr_rhs(hg_t, 0, 1, sl),
                                     start=True, stop=True,
                                     perf_mode=PM.DoubleRow)
                st["e0"] = e0

            def e_x1(st):
                x1_t = st["x1"]
                nc.scalar.activation(x1_t[:, 0, :], st["e0"], AF.Relu,
                                     scale=s0c_t)

            def e_enc1(st):
                e1 = ps.tile([128, TILE], F32, tag="big")
                x1_t = st["x1"]
                for half in range(TILE // 512):
                    sl = slice(half * 512, half * 512 + 512)
                    nc.tensor.matmul(e1[:, sl], we1_t, dr_rhs(x1_t, 0, 1, sl),
                                     start=True, stop=True,
                                     perf_mode=PM.DoubleRow)
                st["e1"] = e1

            def e_x2(st):
                x2_t = st["x2"]
                nc.vector.tensor_scalar(x2_t[:, 0, :], st["e1"], s1c_t, 0.0,
                                        op0=OP.mult, op1=OP.max)

            def e_enc2(st):
                e2 = ps.tile([128, TILE], F32, tag="big")
                x2_t = st["x2"]
                for half in range(TILE // 512):
                    sl = slice(half * 512, half * 512 + 512)
                    nc.tensor.matmul(e2[:, sl], we2_t, dr_rhs(x2_t, 0, 1, sl),
                                     start=True, stop=True,
                                     perf_mode=PM.DoubleRow)
                st["e2"] = e2

            def e_loc(st):
                img, ti = st["img"], st["ti"]
                loc_t = st["loc"]
                e2 = st["e2"]
                # split the evacuation S/V for balance
                nc.scalar.activation(loc_t[:, ti, 0:512], e2[:, 0:512],
                                     AF.Relu, scale=s2c_t)
                nc.vector.tensor_scalar(loc_t[:, ti, 512:TILE],
                                        e2[:, 512:TILE], s2c_t, 0.0,
                                        op0=OP.mult, op1=OP.max)
                nc.vector.reduce_max(out=gparts[:, img, ti:ti + 1],
                                     in_=e2, axis=AX.X)

            E_STAGES = [e_apply, e_bwdT, e_gelu, e_enc0, e_x1, e_enc1, e_x2,
                        e_enc2, e_loc]

            def glob_s0(sh):
                img = sh["img"]
                graw = wp.tile([128, 1], F32, tag="graw")
                nc.vector.reduce_max(out=graw, in_=gparts[:, img], axis=AX.X)
                gbf = wp.tile([128, 1], BF16, tag="gbf")
                nc.scalar.activation(gbf, graw, AF.Relu, scale=s2c_t)
                gmm = psb.tile([128, NCH], F32, tag="small")
                nc.tensor.matmul(gmm[:, 0:1], w0b_t, gbf,
                                 start=True, stop=True)
                b0h = wp.tile([128, 1], F32, tag="b0h")
                nc.vector.tensor_scalar(b0h, gmm[:, 0:1], bh0c_t, None,
                                        op0=OP.add)
                sh["b0h"] = b0h

            def h_h0(st):
                h0 = ps.tile([128, TILE], F32, tag="big")
                loc_t = st["loc"]
                ti = st["ti"]
                for half in range(TILE // 512):
                    sl = slice(half * 512, half * 512 + 512)
                    nc.tensor.matmul(h0[:, sl], w0a_t,
                                     dr_rhs(loc_t, ti, NT, sl),
                                     start=True, stop=True,
                                     perf_mode=PM.DoubleRow)
                st["h0"] = h0

            def h_y0(st):
                y0_t = st["y0"]
                nc.scalar.activation(y0_t[:, 0, :], st["h0"], AF.Relu,
                                     bias=st["sh"]["b0h"], scale=s3c_t)

            def h_h1(st):
                h1 = ps.tile([64, TILE], F32, tag="big")
                y0_t = st["y0"]
                for half in range(TILE // 512):
                    sl = slice(half * 512, half * 512 + 512)
                    nc.tensor.matmul(h1[:, sl], wh1_t, dr_rhs(y0_t, 0, 1, sl),
                                     start=True, stop=True,
                                     perf_mode=PM.DoubleRow)
                st["h1"] = h1

            def h_y1(st):
                y1_t = st["y1"]
                nc.vector.tensor_scalar(y1_t, st["h1"], s4c_t, 0.0,
                                        op0=OP.mult, op1=OP.max)

            def h_dots(st):
                wz = psb.tile([128, NCH], F32, tag="small")
                y1_t = st["y1"]
                for j in range(NCH):
                    nc.tensor.matmul(wz[:, j:j + 1],
                                     y1_t[:, j * 128:(j + 1) * 128],
                                     w2col_t, start=True, stop=True)
                st["wz"] = wz

            def h_w(st):
                img, ti = st["img"], st["ti"]
                wt = wp.tile([128, NCH], F32, tag="wt")
                nc.scalar.activation(wt, st["wz"], AF.Tanh, bias=tb2_t,
                                     scale=0.5)
                nc.vector.tensor_scalar(
                    w_all[:, img, ti * NCH:(ti + 1) * NCH], wt, 0.5, 0.5,
                    op0=OP.mult, op1=OP.add)

            H_STAGES = [h_h0, h_y0, h_h1, h_y1, h_dots, h_w]

            def gram_img(st):
                img = st["img"]
                q = qh_sb[:, img]
                qw_t = wp.tile([128, 9, NC32], F32, tag="qw")
                wv = w_all[:, img]
                w_bc = bass.AP(tensor=wv.tensor, offset=wv.offset,
                               ap=[wv.ap[0], [0, 9], wv.ap[-1]])
                nc.gpsimd.tensor_tensor(out=qw_t, in0=q, in1=w_bc,
                                        op=OP.mult)
                gm_ps = psb.tile([9, 9], F32, tag="small")
                for c in range(NC32):
                    nc.tensor.matmul(gm_ps, qw_t[:, :, c], q[:, :, c],
                                     start=(c == 0), stop=(c == NC32 - 1))
                gm_sb = wp.tile([9, 9], F32, tag="gms")
                nc.vector.tensor_copy(gm_sb, gm_ps)
                nc.sync.dma_start(out=out[img], in_=gm_sb)

            # ---------------- schedule ----------------
            def run_window(units, W=2):
                active = []
                idx = 0
                while idx < len(units) or active:
                    while len(active) < W and idx < len(units):
                        stages, st = units[idx]
                        active.append([stages, st, 0])
                        idx += 1
                    for u in list(active):
                        stages, st, k = u
                        stages[k](st)
                        u[2] += 1
                        if u[2] >= len(stages):
                            active.remove(u)

            # phase 1 with deep DMA prefetch
            p1_sts = [{"img": img, "ti": ti, "t": img * NT + ti}
                      for img in range(BL) for ti in range(NT)]
            for st in p1_sts[:4]:
                p1_load(st)
            for i in range(0, len(p1_sts), 2):
                pair = p1_sts[i:i + 2]
                for st in p1_sts[i + 4:i + 6]:
                    p1_load(st)
                for stg in P1_STAGES:
                    for st in pair:
                        stg(st)

            newton_all()

            units = []
            for img in range(BL):
                sh = {"img": img}
                loc_t = loc_b[img % 2]
                e_units = []
                h_units = []
                for ti in range(NT):
                    k = (img * NT + ti) % 2
                    st = {"img": img, "ti": ti, "t": img * NT + ti,
                          "sh": sh, "hg": hg_b[k], "x1": x1_b[k],
                          "x2": x2_b[k], "y0": y0_b[k], "y1": y1_b[k],
                          "loc": loc_t}
                    e_units.append((E_STAGES, st))
                    h_units.append((H_STAGES, st))
                units += (e_units + [([glob_s0], sh)] + h_units
                          + [([gram_img], {"img": img})])
            run_window(units, W=2)

    nc.compile()
    return nc


_CACHE = {}


def _get_nc():
    if "nc" not in _CACHE:
        _CACHE["nc"] = build()
    return _CACHE["nc"]


def _hartley(pts):
    pts = pts.astype(np.float32)
    centroid = pts.mean(axis=1, keepdims=True)
    pc = pts - centroid
    dist = np.sqrt(np.clip((pc ** 2).sum(-1), 0.0, None))
    mean_dist = dist.mean(axis=1, keepdims=True)
    scale = np.float32(np.sqrt(2.0)) / np.clip(mean_dist, 0.001, None)
    scale = np.where(mean_dist < 0.001, np.ones_like(scale), scale)
    pts_norm = pc * scale[..., None]
    return (pts_norm.astype(np.float32), scale[:, 0].astype(np.float32),
            centroid[:, 0, 0].astype(np.float32),
            centroid[:, 0, 1].astype(np.float32))


def _pow2(x):
    """Largest power of two <= x (elementwise, safe)."""
    x = np.maximum(np.asarray(x, np.float64), 1e-30)
    return np.exp2(np.floor(np.log2(x))).astype(np.float32)


def _rowquant(Wrow_mats, target=120.0):
    """Per-row pow2 scale k for a list of [out, in_i] f32 matrices sharing
    rows. Returns (list of fp8 matrices scaled by k, inv_scale [out] f32)."""
    mx = np.zeros(Wrow_mats[0].shape[0], np.float64)
    for M in Wrow_mats:
        if M.size:
            mx = np.maximum(mx, np.abs(M).max(axis=1))
    k = _pow2(target / np.maximum(mx, 1e-30))
    k = np.clip(k, 2.0 ** -30, 2.0 ** 30).astype(np.float32)
    q = [np.clip(M * k[:, None], -240, 240).astype(E4) for M in Wrow_mats]
    return q, (1.0 / k).astype(np.float32)


def _fp8_bias_rows(b, k):
    """bias*k split into fp8 hi+lo rows."""
    z = np.clip(b * k, -240 * 1.99, 240 * 1.99).astype(np.float64)
    hi = np.clip(z, -240, 240).astype(E4)
    lo = np.clip(z - hi.astype(np.float64), -240, 240).astype(E4)
    return hi, lo


def kernel(pos_A, pos_B, feat_A, feat_B,
           fc_w1, fc_b1, fc_ln_g, fc_ln_b, fc_w2, fc_b2,
           enc_w0, enc_g0, enc_b0, enc_w1, enc_g1, enc_b1,
           enc_w2, enc_g2, enc_b2,
           head_w0, head_g0, head_b0, head_w1, head_g1, head_b1,
           head_w2, head_b2):
    f32 = np.float32
    pos_A = np.asarray(pos_A, f32)
    pos_B = np.asarray(pos_B, f32)
    fA = np.asarray(feat_A, f32)
    fB = np.asarray(feat_B, f32)

    # ---- folded f32 weights ----
    bnsc = f32(1.0 / np.sqrt(1.0 + EPS))
    w1c = (fc_w1 - fc_w1.mean(axis=0, keepdims=True)).astype(f32)
    b1cv = (fc_b1 - fc_b1.mean()).astype(f32)
    enc_w0s = (enc_w0 * (enc_g0 * bnsc)[:, None]).astype(f32)
    enc_w1s = (enc_w1 * (enc_g1 * bnsc)[:, None]).astype(f32)
    enc_w2s = (enc_w2 * (enc_g2 * bnsc)[:, None]).astype(f32)
    head_w0s = (head_w0 * (head_g0 * bnsc)[:, None]).astype(f32)
    head_w1s = (head_w1 * (head_g1 * bnsc)[:, None]).astype(f32)
    wfold = (enc_w0s[:, 4:36] @ fc_w2).astype(f32)          # [128, 64]
    benc0 = (enc_b0 + enc_w0s[:, 4:36] @ fc_b2).astype(f32)
    wpos = enc_w0s[:, 0:4].astype(f32)                      # [128, 4]
    w0a_f = head_w0s[:, 0:128]
    w0b_f = head_w0s[:, 128:256]

    # ---- input featurization ----
    d_f = np.abs(fA - fB)                                   # [B, N, C]
    m_f = fA * fB
    ad = float(_pow2(200.0 / max(d_f.max(), 1e-9)))
    am = float(_pow2(200.0 / max(np.abs(m_f).max(), 1e-9)))

    # ---- calibration on two images (f32 forward) ----
    def fwd(img):
        di, mi = d_f[img], m_f[img]
        h = np.concatenate([di, mi], -1) @ w1c.T + b1cv
        var = (h * h).mean(-1, keepdims=True)
        hn = h / np.sqrt(var + EPS) * fc_ln_g + fc_ln_b
        from scipy.special import erf
        hg = hn * 0.5 * (1.0 + erf(hn / np.sqrt(2.0)))
        e0 = hg @ wfold.T + np.concatenate([pos_A[img], pos_B[img]],
                                           -1) @ wpos.T + benc0
        x1 = np.maximum(e0, 0)
        x2 = np.maximum(x1 @ enc_w1s.T + enc_b1, 0)
        loc = np.maximum(x2 @ enc_w2s.T + enc_b2, 0)
        glob = loc.max(axis=0)
        b0h = head_b0 + w0b_f @ glob
        y0 = np.maximum(loc @ w0a_f.T + b0h, 0)
        y1 = np.maximum(y0 @ head_w1s.T + head_b1, 0)
        return (x1.max(0), x2.max(0), loc.max(0), y0.max(0), y1.max(0))

    mxs = [np.maximum(a, b) for a, b in zip(fwd(0), fwd(B // 2))]
    tgt = 48.0
    a1 = _pow2(tgt / np.maximum(mxs[0].max(), 1e-9))
    a2 = _pow2(tgt / np.maximum(mxs[1].max(), 1e-9))
    al = _pow2(tgt / np.maximum(mxs[2].max(), 1e-9))
    a3 = _pow2(tgt / np.maximum(mxs[3].max(), 1e-9))

    # ---- quantized params ----
    (w1d_q, w1m_q), inv1 = _rowquant([w1c[:, 0:128] / ad,
                                      w1c[:, 128:256] / am])
    w1q_h = np.zeros((128, 2, 64), E4)
    w1q_h[:, 0, :] = w1d_q.T
    w1q_h[:, 1, :] = w1m_q.T

    (we0_q,), inv0 = _rowquant(
        [np.concatenate([wfold, wpos * 4, wpos * 4, wpos * 4], axis=1)])
    k0 = 1.0 / inv0
    b0hi, b0lo = _fp8_bias_rows(benc0, k0)
    we0_h = np.zeros((76, 2, 128), E4)
    we0_h[:, 0, :] = we0_q.T
    we0_h[0, 1, :] = b0hi
    we0_h[1, 1, :] = b0lo

    (we1_q,), invs1 = _rowquant([enc_w1s / a1])
    b1hi, b1lo = _fp8_bias_rows(enc_b1, 1.0 / invs1)
    we1_h = np.zeros((128, 2, 128), E4)
    we1_h[:, 0, :] = we1_q.T
    we1_h[0, 1, :] = b1hi
    we1_h[1, 1, :] = b1lo

    (we2_q,), invs2 = _rowquant([enc_w2s / a2])
    b2hi, b2lo = _fp8_bias_rows(enc_b2, 1.0 / invs2)
    we2_h = np.zeros((128, 2, 128), E4)
    we2_h[:, 0, :] = we2_q.T
    we2_h[0, 1, :] = b2hi
    we2_h[1, 1, :] = b2lo

    (w0a_q,), invs3 = _rowquant([w0a_f / al])
    w0a_h = np.zeros((128, 2, 128), E4)
    w0a_h[:, 0, :] = w0a_q.T

    (wh1_q,), invs4 = _rowquant([head_w1s / a3])
    bh1hi, bh1lo = _fp8_bias_rows(head_b1, 1.0 / invs4)
    wh1_h = np.zeros((128, 2, 64), E4)
    wh1_h[:, 0, :] = wh1_q.T
    wh1_h[0, 1, :] = bh1hi
    wh1_h[1, 1, :] = bh1lo

    params = {
        "w1q": w1q_h,
        "sfc": inv1.reshape(64, 1),
        "b1c": b1cv.reshape(64, 1),
        "gcol": fc_ln_g.astype(f32).reshape(64, 1),
        "bln": fc_ln_b.astype(f32).reshape(64, 1),
        "we0": we0_h, "s0c": (inv0 * a1).reshape(128, 1).astype(f32),
        "we1": we1_h, "s1c": (invs1 * a2).reshape(128, 1).astype(f32),
        "we2": we2_h, "s2c": (invs2 * al).reshape(128, 1).astype(f32),
        "w0a": w0a_h, "s3c": (invs3 * a3).reshape(128, 1).astype(f32),
        "w0b": (w0b_f * (a3 / al)).T.astype(BF),
        "bh0c": (head_b0 * a3).reshape(128, 1).astype(f32),
        "wh1": wh1_h, "s4c": invs4.reshape(64, 1).astype(f32),
        "w2col": head_w2.reshape(64, 1).astype(BF),
        "tb2": np.full((128, 1), 0.5 * float(head_b2[0]), f32),
    }

    # zero/ones plane: partitions 0/1 = 1.0, rest 0
    zp = np.zeros((128, TILE), f32)
    zp[0:2] = 1.0
    params["zpl"] = zp.astype(E4)

    # ---- per-core data tensors ----
    # d/m packed [B, 128, 2, N] fp8
    dmh = np.empty((B, C, 2, N), E4)
    dmh[:, :, 0, :] = np.clip(d_f * ad, 0, 240).transpose(0, 2, 1).astype(E4)
    dmh[:, :, 1, :] = np.clip(m_f * am, -240, 240).transpose(0, 2, 1).astype(E4)
    # but the weights were built for d_true = d_stored/ad with stored=d*ad:
    # inv1 built from w/ad so PSUM = k*(w.d_true*ad/ad)... handled above.

    # positions hi/mid/lo fp8, scaled by 1/4
    psc = np.concatenate([pos_A, pos_B], -1).transpose(0, 2, 1) * 0.25
    hi = np.clip(psc, -240, 240).astype(E4)
    r1 = psc - hi.astype(f32)
    mid = np.clip(r1, -240, 240).astype(E4)
    r2 = r1 - mid.astype(f32)
    lo = np.clip(r2, -240, 240).astype(E4)
    posq_h = np.concatenate([hi, mid, lo], axis=1)          # [B, 12, N]

    # q monomials (Hartley-normalized)
    srcn, sA, cxA, cyA = _hartley(pos_A)
    dstn, sB, cxB, cyB = _hartley(pos_B)
    sx, sy = srcn[..., 0], srcn[..., 1]
    dx, dy = dstn[..., 0], dstn[..., 1]
    one = np.ones_like(sx)
    q9 = np.stack([sx, sy, one, dx, dy, dx * sx, dx * sy, dy * sx, dy * sy],
                  axis=-1)                                   # [B, N, 9]
    qh_h = np.ascontiguousarray(
        q9.reshape(B, NC32, 128, 9).transpose(2, 0, 3, 1)).astype(f32)
    # [128, B, 9, NC32]

    in_maps = []
    for i in range(NCORES):
        sl = slice(i * BL, (i + 1) * BL)
        mcore = {"dm": np.ascontiguousarray(dmh[sl]),
                 "posq": np.ascontiguousarray(posq_h[sl]),
                 "qh": np.ascontiguousarray(qh_h[:, sl])}
        mcore.update(params)
        in_maps.append(mcore)

    nc = _get_nc()
    res = bass_utils.run_bass_kernel_spmd(nc, in_maps,
                                          core_ids=list(range(NCORES)))
    M = np.concatenate([res.results[i]["out"] for i in range(NCORES)],
                       axis=0).astype(f32)                  # [B, 9, 9]

    # ---- host post: assemble AtWA/AtWb, solve, compose ----
    u3 = [0, 1, 2]
    AtWA = np.zeros((B, 8, 8), f32)
    AtWA[:, 0:3, 0:3] = M[:, 0:3, 0:3]
    AtWA[:, 3:6, 3:6] = M[:, 0:3, 0:3]
    AtWA[:, 0:3, 6] = -M[:, u3, 5]
    AtWA[:, 0:3, 7] = -M[:, u3, 6]
    AtWA[:, 3:6, 6] = -M[:, u3, 7]
    AtWA[:, 3:6, 7] = -M[:, u3, 8]
    AtWA[:, 6, 0:3] = -M[:, u3, 5]
    AtWA[:, 7, 0:3] = -M[:, u3, 6]
    AtWA[:, 6, 3:6] = -M[:, u3, 7]
    AtWA[:, 7, 3:6] = -M[:, u3, 8]
    AtWA[:, 6, 6] = M[:, 5, 5] + M[:, 7, 7]
    AtWA[:, 6, 7] = M[:, 5, 6] + M[:, 7, 8]
    AtWA[:, 7, 6] = M[:, 6, 5] + M[:, 8, 7]
    AtWA[:, 7, 7] = M[:, 6, 6] + M[:, 8, 8]
    AtWb = np.zeros((B, 8), f32)
    AtWb[:, 0:3] = M[:, 3, 0:3]
    AtWb[:, 3:6] = M[:, 4, 0:3]
    AtWb[:, 6] = -(M[:, 3, 5] + M[:, 4, 7])
    AtWb[:, 7] = -(M[:, 3, 6] + M[:, 4, 8])
    AtWA += REG * np.eye(8, dtype=f32)[None]
    h_id = np.array([1, 0, 0, 0, 1, 0, 0, 0], f32)
    AtWb += REG * h_id[None]

    try:
        h8 = np.linalg.solve(AtWA, AtWb[..., None])[..., 0].astype(f32)
    except np.linalg.LinAlgError:
        h8 = np.zeros((B, 8), f32)
        for b in range(B):
            try:
                h8[b] = np.linalg.solve(AtWA[b], AtWb[b])
            except np.linalg.LinAlgError:
                h8[b] = np.nan
    finite = np.all(np.isfinite(h8), axis=-1, keepdims=True)
    h8 = np.where(finite, h8, h_id[None])
    H_norm = np.concatenate([h8, np.ones((B, 1), f32)], axis=-1)
    H_norm = H_norm.reshape(B, 3, 3)

    T_src = np.zeros((B, 3, 3), f32)
    T_src[:, 0, 0] = sA
    T_src[:, 1, 1] = sA
    T_src[:, 0, 2] = -sA * cxA
    T_src[:, 1, 2] = -sA * cyA
    T_src[:, 2, 2] = 1.0
    s_dst = np.clip(sB, 1e-6, None)
    T_dst_inv = np.zeros((B, 3, 3), f32)
    T_dst_inv[:, 0, 0] = 1.0 / s_dst
    T_dst_inv[:, 1, 1] = 1.0 / s_dst
    T_dst_inv[:, 0, 2] = (sB * cxB) / s_dst
    T_dst_inv[:, 1, 2] = (sB * cyB) / s_dst
    T_dst_inv[:, 2, 2] = 1.0

    H = (T_dst_inv @ (H_norm @ T_src)).astype(f32)
    H = H / np.clip(np.abs(H[:, 2:3, 2:3]), 1e-8, None)
    h33 = H[:, 2:3, 2:3]
    sgn = np.sign(h33)
    sgn = np.where(sgn == 0, np.ones_like(sgn), sgn)
    H = H / (np.clip(np.abs(h33), 1e-8, None) * sgn)
    H_finite = np.all(np.isfinite(H), axis=(-2, -1))
    a33 = np.abs(H[:, 2, 2])
    valid = H_finite & (a33 > 1e-4) & (a33 < 1e4)
    eye = np.eye(3, dtype=f32)
    H = np.where(valid[:, None, None], H, eye[None])
    return H.astype(f32)


# revision 14
# speedup vs baseline: 1.2476x; 1.2476x over previous
"""AgriMatcher Trainium2 kernel: point-matching network + weighted-DLT homography.

Data-parallel over batch B=64 across 8 NeuronCores (8 images/core). The device
runs the network (fc-compression + LayerNorm + gelu, PointNet encoder, weight
head) and accumulates the per-image 9x9 weighted Gram matrix
M = sum_n w_n q_n q_n^T over Hartley-normalized point monomials q (host-built).
Host assembles AtWA/AtWb from M, solves 8x8, composes the 3x3 homographies.

Perf structure:
- fc1 and all five 128-wide layers run as fp8(E4M3) DoubleRow matmuls
  (2 fp8 rows/PE-cell = 2x-4x tensor throughput). Weights carry per-row pow2
  scales, undone by each evacuation's per-partition scale; layer biases ride a
  static ones-row in the DoubleRow zero-plane.
- Host precomputes |fA-fB| and fA*fB (fp8, pow2-scaled), the DLT q monomials,
  and a 3-way fp8 hi/mid/lo split of the positions for the encoder input.
- LayerNorm via PE transposes; variance by fused square+accumulate
  (scalar_tensor_tensor) on Vector; rstd (fast-invsqrt + Newton) on Vector;
  per-chunk rstd apply on GpSimd.
- Evacuations (PSUM->SBUF w/ relu+scale) balanced across Scalar and Vector.
"""

import numpy as np
import ml_dtypes

import concourse.bass as bass
import concourse.mybir as mybir
import concourse.tile as tile
from concourse import bacc, bass_utils
from concourse.masks import make_identity

F32 = mybir.dt.float32
BF16 = mybir.dt.bfloat16
FP8 = mybir.dt.float8e4
I32 = mybir.dt.int32
AF = mybir.ActivationFunctionType
OP = mybir.AluOpType
AX = mybir.AxisListType
PM = mybir.MatmulPerfMode

B, N, C = 64, 4096, 128
HID, COMP = 128, 32
NCORES = 8
BL = B // NCORES          # images per core
TILE = 1024               # points per tile
NT = N // TILE            # tiles per image (4)
NCH = TILE // 128         # 128-pt chunks per tile (8)
NTC = BL * NT             # tiles per core (32)
NC32 = N // 128           # 128-pt chunks per image (32)
EPS = 1e-5
REG = 1e-4
MAGIC = 0x5F3759DF

BF = ml_dtypes.bfloat16
E4 = ml_dtypes.float8_e4m3


def build():
    nc = bacc.Bacc("TRN2", target_bir_lowering=False, debug=False,
                   num_devices=NCORES)

    dm = nc.dram_tensor("dm", [BL, 128, 2, N], FP8, kind="ExternalInput").ap()
    posq = nc.dram_tensor("posq", [BL, 12, N], FP8, kind="ExternalInput").ap()
    qh = nc.dram_tensor("qh", [128, BL, NC32, 9], F32,
                        kind="ExternalInput").ap()
    zpl = nc.dram_tensor("zpl", [128, TILE], FP8, kind="ExternalInput").ap()
    # params
    w1q = nc.dram_tensor("w1q", [128, 2, 64], FP8, kind="ExternalInput").ap()
    sfc = nc.dram_tensor("sfc", [64, 1], F32, kind="ExternalInput").ap()
    b1c = nc.dram_tensor("b1c", [64, 1], F32, kind="ExternalInput").ap()
    gcol = nc.dram_tensor("gcol", [64, 1], F32, kind="ExternalInput").ap()
    bln = nc.dram_tensor("bln", [64, 1], F32, kind="ExternalInput").ap()
    we0 = nc.dram_tensor("we0", [76, 2, 128], FP8, kind="ExternalInput").ap()
    s0c = nc.dram_tensor("s0c", [128, 1], F32, kind="ExternalInput").ap()
    we1 = nc.dram_tensor("we1", [128, 2, 128], FP8, kind="ExternalInput").ap()
    s1c = nc.dram_tensor("s1c", [128, 1], F32, kind="ExternalInput").ap()
    we2 = nc.dram_tensor("we2", [128, 2, 128], FP8, kind="ExternalInput").ap()
    s2c = nc.dram_tensor("s2c", [128, 1], F32, kind="ExternalInput").ap()
    w0a = nc.dram_tensor("w0a", [128, 2, 128], FP8, kind="ExternalInput").ap()
    s3c = nc.dram_tensor("s3c", [128, 1], F32, kind="ExternalInput").ap()
    w0b = nc.dram_tensor("w0b", [128, 128], BF16, kind="ExternalInput").ap()
    bh0c = nc.dram_tensor("bh0c", [128, 1], F32, kind="ExternalInput").ap()
    wh1 = nc.dram_tensor("wh1", [128, 2, 64], FP8, kind="ExternalInput").ap()
    s4c = nc.dram_tensor("s4c", [64, 1], F32, kind="ExternalInput").ap()
    w2col = nc.dram_tensor("w2col", [64, 1], BF16, kind="ExternalInput").ap()
    tb2 = nc.dram_tensor("tb2", [128, 1], F32, kind="ExternalInput").ap()

    out = nc.dram_tensor("out", [BL, 9, 9], F32, kind="ExternalOutput").ap()

    with tile.TileContext(nc) as tc:
        with (
            tc.tile_pool(name="const", bufs=1) as cp,
            tc.tile_pool(name="persist", bufs=1) as pp,
            tc.tile_pool(name="work", bufs=3) as wp,
            tc.tile_pool(name="feat", bufs=4) as fp,
            tc.tile_pool(name="ps", bufs=2, space="PSUM") as ps,
            tc.tile_pool(name="psb", bufs=2, space="PSUM") as psb,
        ):
            ident = cp.tile([128, 128], BF16)
            make_identity(nc, ident)

            def cload(ap_in, shape, dtype):
                t = cp.tile(shape, dtype, tag=ap_in.tensor.name)
                nc.sync.dma_start(out=t, in_=ap_in)
                return t

            w1q_t = cload(w1q, [128, 2, 64], FP8)
            sfc_t = cload(sfc, [64, 1], F32)
            b1c_t = cload(b1c, [64, 1], F32)
            gcol_t = cload(gcol, [64, 1], F32)
            bln_t = cload(bln, [64, 1], F32)
            we0_t = cload(we0, [76, 2, 128], FP8)
            s0c_t = cload(s0c, [128, 1], F32)
            we1_t = cload(we1, [128, 2, 128], FP8)
            s1c_t = cload(s1c, [128, 1], F32)
            we2_t = cload(we2, [128, 2, 128], FP8)
            s2c_t = cload(s2c, [128, 1], F32)
            w0a_t = cload(w0a, [128, 2, 128], FP8)
            s3c_t = cload(s3c, [128, 1], F32)
            w0b_t = cload(w0b, [128, 128], BF16)
            bh0c_t = cload(bh0c, [128, 1], F32)
            wh1_t = cload(wh1, [128, 2, 64], FP8)
            s4c_t = cload(s4c, [64, 1], F32)
            w2col_t = cload(w2col, [64, 1], BF16)
            tb2_t = cload(tb2, [128, 1], F32)

            qh_sb = pp.tile([128, BL, NC32, 9], F32)
            nc.sync.dma_start(out=qh_sb, in_=qh)

            # persistent state
            hc_all = pp.tile([128, NTC, NCH, 64], BF16)
            s2_all = pp.tile([128, NTC, NCH], F32)
            rstd_all = pp.tile([128, NTC * NCH], F32)
            rstd_bf = pp.tile([128, NTC * NCH], BF16)
            vp_all = pp.tile([128, NTC * NCH], F32)
            u_all = pp.tile([128, NTC * NCH], F32)
            w_all = pp.tile([128, BL, NC32], F32)
            gparts = pp.tile([128, BL, NT], F32)

            # fp8 activation tiles: [*, 2, TILE], plane 1 = zero pad with
            # ones at partitions 0/1 (DoubleRow bias rows). DMA'd once.
            hg_b = [pp.tile([76, 2, TILE], FP8, tag=f"hg{i}", name=f"hg{i}")
                    for i in range(2)]
            x1_b = [pp.tile([128, 2, TILE], FP8, tag=f"x1{i}", name=f"x1{i}")
                    for i in range(2)]
            x2_b = [pp.tile([128, 2, TILE], FP8, tag=f"x2{i}", name=f"x2{i}")
                    for i in range(2)]
            y0_b = [pp.tile([128, 2, TILE], FP8, tag=f"y0{i}", name=f"y0{i}")
                    for i in range(2)]
            # local: plane 0..3 = data tiles, plane 4 = zero pad
            loc_b = [pp.tile([128, NT + 1, TILE], FP8, tag=f"lc{i}",
                             name=f"lc{i}") for i in range(2)]
            y1_b = [pp.tile([64, TILE], BF16, tag=f"y1{i}", name=f"y1{i}")
                    for i in range(2)]

            for t in hg_b:
                nc.sync.dma_start(out=t[0:76, 1, :], in_=zpl[0:76, :])
            for t in x1_b + x2_b + y0_b:
                nc.sync.dma_start(out=t[:, 1, :], in_=zpl)
            for t in loc_b:
                nc.sync.dma_start(out=t[:, NT, :], in_=zpl)

            def dr_rhs(t, plane, zplane, sl):
                base = t[:, plane, sl]
                return bass.AP(tensor=base.tensor, offset=base.offset,
                               ap=[base.ap[0],
                                   [(zplane - plane) * TILE, 2],
                                   base.ap[-1]])

            # ---------------- phase 1: fc1 + LN stats ----------------
            def p1_load(st):
                img, ti = st["img"], st["ti"]
                p0 = ti * TILE
                dm_t = fp.tile([128, 2, TILE], FP8, tag="dm")
                nc.sync.dma_start(out=dm_t, in_=dm[img, :, :, p0:p0 + TILE])
                st["dm"] = dm_t

            def p1_fc1(st):
                h_ps = ps.tile([64, TILE], F32, tag="big")
                dm_t = st["dm"]
                for half in range(TILE // 512):
                    sl = slice(half * 512, half * 512 + 512)
                    nc.tensor.matmul(h_ps[:, sl], w1q_t, dm_t[:, :, sl],
                                     start=True, stop=True,
                                     perf_mode=PM.DoubleRow)
                st["h_ps"] = h_ps

            def p1_evac(st):
                h_sb = wp.tile([64, TILE], BF16, tag="h_sb")
                nc.scalar.activation(h_sb, st["h_ps"], AF.Identity,
                                     bias=b1c_t, scale=sfc_t)
                st["h_sb"] = h_sb

            def p1_fwdT(st):
                hp_ps = psb.tile([128, NCH, 64], BF16, tag="tp")
                h_sb = st["h_sb"]
                for j in range(NCH):
                    nc.tensor.transpose(hp_ps[:, j, :],
                                        h_sb[:, j * 128:(j + 1) * 128],
                                        ident[:64, :64])
                st["hp_ps"] = hp_ps

            def p1_sq(st):
                t = st["t"]
                hp = st["hp_ps"]
                nc.vector.tensor_copy(
                    hc_all[:, t].rearrange("p a b -> p (a b)"),
                    hp.rearrange("p a b -> p (a b)"))
                sqd = wp.tile([128, NCH, 64], BF16, tag="sqd")
                hc = hc_all[:, t]
                for j in range(NCH):
                    nc.vector.scalar_tensor_tensor(
                        out=sqd[:, j], in0=hc[:, j], scalar=0.0,
                        in1=hc[:, j], op0=OP.bypass, op1=OP.mult,
                        accum_out=s2_all[:, t, j:j + 1])

            P1_STAGES = [p1_fc1, p1_evac, p1_fwdT, p1_sq]

            def newton_all():
                s2f = s2_all.rearrange("p a b -> p (a b)")
                vp, yv, u_t = vp_all, rstd_all, u_all
                nc.vector.tensor_scalar(vp, s2f, 1.0 / 64.0, EPS,
                                        op0=OP.mult, op1=OP.add)
                nc.vector.tensor_scalar(yv.bitcast(I32), vp.bitcast(I32), 1,
                                        None, op0=OP.arith_shift_right)
                nc.vector.tensor_scalar(yv.bitcast(I32), yv.bitcast(I32),
                                        0xFFFFFFFF, None, op0=OP.bitwise_xor)
                nc.vector.tensor_scalar(yv.bitcast(I32), yv.bitcast(I32),
                                        MAGIC + 1, None, op0=OP.add)
                for _ in range(3):
                    nc.vector.tensor_mul(u_t, yv, yv)
                    nc.vector.tensor_mul(u_t, u_t, vp)
                    nc.vector.tensor_scalar(u_t, u_t, -0.5, 1.5,
                                            op0=OP.mult, op1=OP.add)
                    nc.vector.tensor_mul(yv, yv, u_t)
                nc.vector.tensor_copy(rstd_bf, rstd_all)

            # ---------------- phase 2 stages ----------------
            def e_apply(st):
                t = st["t"]
                hcn = wp.tile([128, NCH, 64], BF16, tag="hcn")
                rb = rstd_bf[:, t * NCH:(t + 1) * NCH]
                rb_bc = bass.AP(tensor=rb.tensor, offset=rb.offset,
                                ap=[rb.ap[0], rb.ap[-1], [0, 64]])
                nc.vector.tensor_tensor(out=hcn, in0=hc_all[:, t],
                                        in1=rb_bc, op=OP.mult)
                st["hcn"] = hcn
                # prefetch positions into the hg buffer
                img, ti = st["img"], st["ti"]
                p0 = ti * TILE
                hg_t = st["hg"]
                nc.sync.dma_start(out=hg_t[64:76, 0, :],
                                  in_=posq[img, :, p0:p0 + TILE])

            def e_bwdT(st):
                ycm = psb.tile([64, TILE], BF16, tag="tp")
                hcn = st["hcn"]
                for j in range(NCH):
                    nc.tensor.transpose(ycm[:, j * 128:(j + 1) * 128],
                                        hcn[:, j], ident)
                st["ycm"] = ycm

            def e_gelu(st):
                hg_t = st["hg"]
                nc.scalar.activation(hg_t[0:64, 0, :], st["ycm"], AF.Gelu,
                                     bias=bln_t, scale=gcol_t)

            def e_enc0(st):
                e0 = ps.tile([128, TILE], F32, tag="big")
                hg_t = st["hg"]
                for half in range(TILE // 512):
                    sl = slice(half * 512, half * 512 + 512)
                    nc.tensor.matmul(e0[:, sl], we0_t,
                                     dr_rhs(hg_t, 0, 1, sl),
                                     start=True, stop=True,
                                     perf_mode=PM.DoubleRow)
                st["e0"] = e0

            def e_x1(st):
                x1_t = st["x1"]
                nc.scalar.activation(x1_t[:, 0, :], st["e0"], AF.Relu,
                                     scale=s0c_t)

            def e_enc1(st):
                e1 = ps.tile([128, TILE], F32, tag="big")
                x1_t = st["x1"]
                for half in range(TILE // 512):
                    sl = slice(half * 512, half * 512 + 512)
                    nc.tensor.matmul(e1[:, sl], we1_t, dr_rhs(x1_t, 0, 1, sl),
                                     start=True, stop=True,
                                     perf_mode=PM.DoubleRow)
                st["e1"] = e1

            def e_x2(st):
                x2_t = st["x2"]
                nc.vector.tensor_scalar(x2_t[:, 0, :], st["e1"], s1c_t, 0.0,
                                        op0=OP.mult, op1=OP.max)

            def e_enc2(st):
                e2 = ps.tile([128, TILE], F32, tag="big")
                x2_t = st["x2"]
                for half in range(TILE // 512):
                    sl = slice(half * 512, half * 512 + 512)
                    nc.tensor.matmul(e2[:, sl], we2_t, dr_rhs(x2_t, 0, 1, sl),
                                     start=True, stop=True,
                                     perf_mode=PM.DoubleRow)
                st["e2"] = e2

            def e_loc(st):
                img, ti = st["img"], st["ti"]
                loc_t = st["loc"]
                e2 = st["e2"]
                # split the evacuation S/V for balance
                nc.scalar.activation(loc_t[:, ti, 0:512], e2[:, 0:512],
                                     AF.Relu, scale=s2c_t)
                nc.vector.tensor_scalar(loc_t[:, ti, 512:TILE],
                                        e2[:, 512:TILE], s2c_t, 0.0,
                                        op0=OP.mult, op1=OP.max)
                nc.vector.reduce_max(out=gparts[:, img, ti:ti + 1],
                                     in_=e2, axis=AX.X)

            E_STAGES = [e_apply, e_bwdT, e_gelu, e_enc0, e_x1, e_enc1, e_x2,
                        e_enc2, e_loc]

            def glob_s0(sh):
                img = sh["img"]
                graw = wp.tile([128, 1], F32, tag="graw")
                nc.vector.reduce_max(out=graw, in_=gparts[:, img], axis=AX.X)
                gbf = wp.tile([128, 1], BF16, tag="gbf")
                nc.scalar.activation(gbf, graw, AF.Relu, scale=s2c_t)
                gmm = psb.tile([128, NCH], F32, tag="small")
                nc.tensor.matmul(gmm[:, 0:1], w0b_t, gbf,
                                 start=True, stop=True)
                b0h = wp.tile([128, 1], F32, tag="b0h")
                nc.vector.tensor_scalar(b0h, gmm[:, 0:1], bh0c_t, None,
                                        op0=OP.add)
                sh["b0h"] = b0h

            def h_h0(st):
                h0 = ps.tile([128, TILE], F32, tag="big")
                loc_t = st["loc"]
                ti = st["ti"]
                for half in range(TILE // 512):
                    sl = slice(half * 512, half * 512 + 512)
                    nc.tensor.matmul(h0[:, sl], w0a_t,
                                     dr_rhs(loc_t, ti, NT, sl),
                                     start=True, stop=True,
                                     perf_mode=PM.DoubleRow)
                st["h0"] = h0

            def h_y0(st):
                y0_t = st["y0"]
                nc.scalar.activation(y0_t[:, 0, :], st["h0"], AF.Relu,
                                     bias=st["sh"]["b0h"], scale=s3c_t)

            def h_h1(st):
                h1 = ps.tile([64, TILE], F32, tag="big")
                y0_t = st["y0"]
                for half in range(TILE // 512):
                    sl = slice(half * 512, half * 512 + 512)
                    nc.tensor.matmul(h1[:, sl], wh1_t, dr_rhs(y0_t, 0, 1, sl),
                                     start=True, stop=True,
                                     perf_mode=PM.DoubleRow)
                st["h1"] = h1

            def h_y1(st):
                y1_t = st["y1"]
                nc.vector.tensor_scalar(y1_t, st["h1"], s4c_t, 0.0,
                                        op0=OP.mult, op1=OP.max)

            def h_dots(st):
                wz = psb.tile([128, NCH], F32, tag="small")
                y1_t = st["y1"]
                for j in range(NCH):
                    nc.tensor.matmul(wz[:, j:j + 1],
                                     y1_t[:, j * 128:(j + 1) * 128],
                                     w2col_t, start=True, stop=True)
                st["wz"] = wz

            def h_w(st):
                img, ti = st["img"], st["ti"]
                wt = wp.tile([128, NCH], F32, tag="wt")
                nc.scalar.activation(wt, st["wz"], AF.Tanh, bias=tb2_t,
                                     scale=0.5)
                nc.vector.tensor_scalar(
                    w_all[:, img, ti * NCH:(ti + 1) * NCH], wt, 0.5, 0.5,
                    op0=OP.mult, op1=OP.add)

            H_STAGES = [h_h0, h_y0, h_h1, h_y1, h_dots, h_w]

            def gram_img(st):
                img = st["img"]
                q = qh_sb[:, img]
                qw_t = wp.tile([128, NC32, 9], F32, tag="qw")
                wv = w_all[:, img]
                w_bc = bass.AP(tensor=wv.tensor, offset=wv.offset,
                               ap=[wv.ap[0], wv.ap[-1], [0, 9]])
                nc.gpsimd.tensor_tensor(out=qw_t, in0=q, in1=w_bc,
                                        op=OP.mult)
                gm_ps = psb.tile([9, 9], F32, tag="small")
                for c in range(NC32):
                    nc.tensor.matmul(gm_ps, qw_t[:, c], q[:, c],
                                     start=(c == 0), stop=(c == NC32 - 1))
                gm_sb = wp.tile([9, 9], F32, tag="gms")
                nc.vector.tensor_copy(gm_sb, gm_ps)
                nc.sync.dma_start(out=out[img], in_=gm_sb)

            # ---------------- schedule ----------------
            def run_window(units, W=2):
                active = []
                idx = 0
                while idx < len(units) or active:
                    while len(active) < W and idx < len(units):
                        stages, st = units[idx]
                        active.append([stages, st, 0])
                        idx += 1
                    for u in list(active):
                        stages, st, k = u
                        stages[k](st)
                        u[2] += 1
                        if u[2] >= len(stages):
                            active.remove(u)

            # phase 1 with deep DMA prefetch
            p1_sts = [{"img": img, "ti": ti, "t": img * NT + ti}
                      for img in range(BL) for ti in range(NT)]
            for st in p1_sts[:4]:
                p1_load(st)
            for i in range(0, len(p1_sts), 2):
                pair = p1_sts[i:i + 2]
                for st in p1_sts[i + 4:i + 6]:
                    p1_load(st)
                for stg in P1_STAGES:
                    for st in pair:
                        stg(st)

            newton_all()

            # software-pipeline across images: enc(img) runs alongside
            # head(img-1) so the per-image glob barrier never drains the
            # window.
            shs = [{"img": img} for img in range(BL)]
            e_units = {img: [] for img in range(BL)}
            h_units = {img: [] for img in range(BL)}
            for img in range(BL):
                loc_t = loc_b[img % 2]
                for ti in range(NT):
                    k = (img * NT + ti) % 2
                    st = {"img": img, "ti": ti, "t": img * NT + ti,
                          "sh": shs[img], "hg": hg_b[k], "x1": x1_b[k],
                          "x2": x2_b[k], "y0": y0_b[k], "y1": y1_b[k],
                          "loc": loc_t}
                    e_units[img].append((E_STAGES, st))
                    h_units[img].append((H_STAGES, st))
            units = []
            for img in range(BL + 1):
                for ti in range(NT):
                    if img < BL:
                        units.append(e_units[img][ti])
                    if img >= 1:
                        units.append(h_units[img - 1][ti])
                if img < BL:
                    units.append(([glob_s0], shs[img]))
                if img >= 1:
                    units.append(([gram_img], {"img": img - 1}))
            run_window(units, W=2)

    nc.compile()
    return nc


_CACHE = {}


def _get_nc():
    if "nc" not in _CACHE:
        _CACHE["nc"] = build()
    return _CACHE["nc"]


def _hartley(pts):
    pts = pts.astype(np.float32)
    centroid = pts.mean(axis=1, keepdims=True)
    pc = pts - centroid
    dist = np.sqrt(np.clip((pc ** 2).sum(-1), 0.0, None))
    mean_dist = dist.mean(axis=1, keepdims=True)
    scale = np.float32(np.sqrt(2.0)) / np.clip(mean_dist, 0.001, None)
    scale = np.where(mean_dist < 0.001, np.ones_like(scale), scale)
    pts_norm = pc * scale[..., None]
    return (pts_norm.astype(np.float32), scale[:, 0].astype(np.float32),
            centroid[:, 0, 0].astype(np.float32),
            centroid[:, 0, 1].astype(np.float32))


def _pow2(x):
    """Largest power of two <= x (elementwise, safe)."""
    x = np.maximum(np.asarray(x, np.float64), 1e-30)
    return np.exp2(np.floor(np.log2(x))).astype(np.float32)


def _rowquant(Wrow_mats, target=120.0):
    """Per-row pow2 scale k for a list of [out, in_i] f32 matrices sharing
    rows. Returns (list of fp8 matrices scaled by k, inv_scale [out] f32)."""
    mx = np.zeros(Wrow_mats[0].shape[0], np.float64)
    for M in Wrow_mats:
        if M.size:
            mx = np.maximum(mx, np.abs(M).max(axis=1))
    k = _pow2(target / np.maximum(mx, 1e-30))
    k = np.clip(k, 2.0 ** -30, 2.0 ** 30).astype(np.float32)
    q = [np.clip(M * k[:, None], -240, 240).astype(E4) for M in Wrow_mats]
    return q, (1.0 / k).astype(np.float32)


def _fp8_bias_rows(b, k):
    """bias*k split into fp8 hi+lo rows."""
    z = np.clip(b * k, -240 * 1.99, 240 * 1.99).astype(np.float64)
    hi = np.clip(z, -240, 240).astype(E4)
    lo = np.clip(z - hi.astype(np.float64), -240, 240).astype(E4)
    return hi, lo


def kernel(pos_A, pos_B, feat_A, feat_B,
           fc_w1, fc_b1, fc_ln_g, fc_ln_b, fc_w2, fc_b2,
           enc_w0, enc_g0, enc_b0, enc_w1, enc_g1, enc_b1,
           enc_w2, enc_g2, enc_b2,
           head_w0, head_g0, head_b0, head_w1, head_g1, head_b1,
           head_w2, head_b2):
    f32 = np.float32
    pos_A = np.asarray(pos_A, f32)
    pos_B = np.asarray(pos_B, f32)
    fA = np.asarray(feat_A, f32)
    fB = np.asarray(feat_B, f32)

    # ---- folded f32 weights ----
    bnsc = f32(1.0 / np.sqrt(1.0 + EPS))
    w1c = (fc_w1 - fc_w1.mean(axis=0, keepdims=True)).astype(f32)
    b1cv = (fc_b1 - fc_b1.mean()).astype(f32)
    enc_w0s = (enc_w0 * (enc_g0 * bnsc)[:, None]).astype(f32)
    enc_w1s = (enc_w1 * (enc_g1 * bnsc)[:, None]).astype(f32)
    enc_w2s = (enc_w2 * (enc_g2 * bnsc)[:, None]).astype(f32)
    head_w0s = (head_w0 * (head_g0 * bnsc)[:, None]).astype(f32)
    head_w1s = (head_w1 * (head_g1 * bnsc)[:, None]).astype(f32)
    wfold = (enc_w0s[:, 4:36] @ fc_w2).astype(f32)          # [128, 64]
    benc0 = (enc_b0 + enc_w0s[:, 4:36] @ fc_b2).astype(f32)
    wpos = enc_w0s[:, 0:4].astype(f32)                      # [128, 4]
    w0a_f = head_w0s[:, 0:128]
    w0b_f = head_w0s[:, 128:256]

    # ---- input featurization ----
    d_f = np.abs(fA - fB)                                   # [B, N, C]
    m_f = fA * fB
    ad = float(_pow2(200.0 / max(d_f.max(), 1e-9)))
    am = float(_pow2(200.0 / max(np.abs(m_f).max(), 1e-9)))

    # ---- calibration on two images (f32 forward) ----
    def fwd(img):
        di, mi = d_f[img], m_f[img]
        h = np.concatenate([di, mi], -1) @ w1c.T + b1cv
        var = (h * h).mean(-1, keepdims=True)
        hn = h / np.sqrt(var + EPS) * fc_ln_g + fc_ln_b
        from scipy.special import erf
        hg = hn * 0.5 * (1.0 + erf(hn / np.sqrt(2.0)))
        e0 = hg @ wfold.T + np.concatenate([pos_A[img], pos_B[img]],
                                           -1) @ wpos.T + benc0
        x1 = np.maximum(e0, 0)
        x2 = np.maximum(x1 @ enc_w1s.T + enc_b1, 0)
        loc = np.maximum(x2 @ enc_w2s.T + enc_b2, 0)
        glob = loc.max(axis=0)
        b0h = head_b0 + w0b_f @ glob
        y0 = np.maximum(loc @ w0a_f.T + b0h, 0)
        y1 = np.maximum(y0 @ head_w1s.T + head_b1, 0)
        return (x1.max(0), x2.max(0), loc.max(0), y0.max(0), y1.max(0))

    mxs = [np.maximum(a, b) for a, b in zip(fwd(0), fwd(B // 2))]
    tgt = 48.0
    a1 = _pow2(tgt / np.maximum(mxs[0].max(), 1e-9))
    a2 = _pow2(tgt / np.maximum(mxs[1].max(), 1e-9))
    al = _pow2(tgt / np.maximum(mxs[2].max(), 1e-9))
    a3 = _pow2(tgt / np.maximum(mxs[3].max(), 1e-9))

    # ---- quantized params ----
    (w1d_q, w1m_q), inv1 = _rowquant([w1c[:, 0:128] / ad,
                                      w1c[:, 128:256] / am])
    w1q_h = np.zeros((128, 2, 64), E4)
    w1q_h[:, 0, :] = w1d_q.T
    w1q_h[:, 1, :] = w1m_q.T

    (we0_q,), inv0 = _rowquant(
        [np.concatenate([wfold, wpos * 4, wpos * 4, wpos * 4], axis=1)])
    k0 = 1.0 / inv0
    b0hi, b0lo = _fp8_bias_rows(benc0, k0)
    we0_h = np.zeros((76, 2, 128), E4)
    we0_h[:, 0, :] = we0_q.T
    we0_h[0, 1, :] = b0hi
    we0_h[1, 1, :] = b0lo

    (we1_q,), invs1 = _rowquant([enc_w1s / a1])
    b1hi, b1lo = _fp8_bias_rows(enc_b1, 1.0 / invs1)
    we1_h = np.zeros((128, 2, 128), E4)
    we1_h[:, 0, :] = we1_q.T
    we1_h[0, 1, :] = b1hi
    we1_h[1, 1, :] = b1lo

    (we2_q,), invs2 = _rowquant([enc_w2s / a2])
    b2hi, b2lo = _fp8_bias_rows(enc_b2, 1.0 / invs2)
    we2_h = np.zeros((128, 2, 128), E4)
    we2_h[:, 0, :] = we2_q.T
    we2_h[0, 1, :] = b2hi
    we2_h[1, 1, :] = b2lo

    (w0a_q,), invs3 = _rowquant([w0a_f / al])
    w0a_h = np.zeros((128, 2, 128), E4)
    w0a_h[:, 0, :] = w0a_q.T

    (wh1_q,), invs4 = _rowquant([head_w1s / a3])
    bh1hi, bh1lo = _fp8_bias_rows(head_b1, 1.0 / invs4)
    wh1_h = np.zeros((128, 2, 64), E4)
    wh1_h[:, 0, :] = wh1_q.T
    wh1_h[0, 1, :] = bh1hi
    wh1_h[1, 1, :] = bh1lo

    params = {
        "w1q": w1q_h,
        "sfc": inv1.reshape(64, 1),
        "b1c": b1cv.reshape(64, 1),
        "gcol": fc_ln_g.astype(f32).reshape(64, 1),
        "bln": fc_ln_b.astype(f32).reshape(64, 1),
        "we0": we0_h, "s0c": (inv0 * a1).reshape(128, 1).astype(f32),
        "we1": we1_h, "s1c": (invs1 * a2).reshape(128, 1).astype(f32),
        "we2": we2_h, "s2c": (invs2 * al).reshape(128, 1).astype(f32),
        "w0a": w0a_h, "s3c": (invs3 * a3).reshape(128, 1).astype(f32),
        "w0b": (w0b_f * (a3 / al)).T.astype(BF),
        "bh0c": (head_b0 * a3).reshape(128, 1).astype(f32),
        "wh1": wh1_h, "s4c": invs4.reshape(64, 1).astype(f32),
        "w2col": head_w2.reshape(64, 1).astype(BF),
        "tb2": np.full((128, 1), 0.5 * float(head_b2[0]), f32),
    }

    # zero/ones plane: partitions 0/1 = 1.0, rest 0
    zp = np.zeros((128, TILE), f32)
    zp[0:2] = 1.0
    params["zpl"] = zp.astype(E4)

    # ---- per-core data tensors ----
    # d/m packed [B, 128, 2, N] fp8
    dmh = np.empty((B, C, 2, N), E4)
    dmh[:, :, 0, :] = np.clip(d_f * ad, 0, 240).transpose(0, 2, 1).astype(E4)
    dmh[:, :, 1, :] = np.clip(m_f * am, -240, 240).transpose(0, 2, 1).astype(E4)
    # but the weights were built for d_true = d_stored/ad with stored=d*ad:
    # inv1 built from w/ad so PSUM = k*(w.d_true*ad/ad)... handled above.

    # positions hi/mid/lo fp8, scaled by 1/4
    psc = np.concatenate([pos_A, pos_B], -1).transpose(0, 2, 1) * 0.25
    hi = np.clip(psc, -240, 240).astype(E4)
    r1 = psc - hi.astype(f32)
    mid = np.clip(r1, -240, 240).astype(E4)
    r2 = r1 - mid.astype(f32)
    lo = np.clip(r2, -240, 240).astype(E4)
    posq_h = np.concatenate([hi, mid, lo], axis=1)          # [B, 12, N]

    # q monomials (Hartley-normalized)
    srcn, sA, cxA, cyA = _hartley(pos_A)
    dstn, sB, cxB, cyB = _hartley(pos_B)
    sx, sy = srcn[..., 0], srcn[..., 1]
    dx, dy = dstn[..., 0], dstn[..., 1]
    one = np.ones_like(sx)
    q9 = np.stack([sx, sy, one, dx, dy, dx * sx, dx * sy, dy * sx, dy * sy],
                  axis=-1)                                   # [B, N, 9]
    qh_h = np.ascontiguousarray(
        q9.reshape(B, NC32, 128, 9).transpose(2, 0, 1, 3)).astype(f32)
    # [128, B, NC32, 9]

    in_maps = []
    for i in range(NCORES):
        sl = slice(i * BL, (i + 1) * BL)
        mcore = {"dm": np.ascontiguousarray(dmh[sl]),
                 "posq": np.ascontiguousarray(posq_h[sl]),
                 "qh": np.ascontiguousarray(qh_h[:, sl])}
        mcore.update(params)
        in_maps.append(mcore)

    nc = _get_nc()
    res = bass_utils.run_bass_kernel_spmd(nc, in_maps,
                                          core_ids=list(range(NCORES)))
    M = np.concatenate([res.results[i]["out"] for i in range(NCORES)],
                       axis=0).astype(f32)                  # [B, 9, 9]

    # ---- host post: assemble AtWA/AtWb, solve, compose ----
    u3 = [0, 1, 2]
    AtWA = np.zeros((B, 8, 8), f32)
    AtWA[:, 0:3, 0:3] = M[:, 0:3, 0:3]
    AtWA[:, 3:6, 3:6] = M[:, 0:3, 0:3]
    AtWA[:, 0:3, 6] = -M[:, u3, 5]
    AtWA[:, 0:3, 7] = -M[:, u3, 6]
    AtWA[:, 3:6, 6] = -M[:, u3, 7]
    AtWA[:, 3:6, 7] = -M[:, u3, 8]
    AtWA[:, 6, 0:3] = -M[:, u3, 5]
    AtWA[:, 7, 0:3] = -M[:, u3, 6]
    AtWA[:, 6, 3:6] = -M[:, u3, 7]
    AtWA[:, 7, 3:6] = -M[:, u3, 8]
    AtWA[:, 6, 6] = M[:, 5, 5] + M[:, 7, 7]
    AtWA[:, 6, 7] = M[:, 5, 6] + M[:, 7, 8]
    AtWA[:, 7, 6] = M[:, 6, 5] + M[:, 8, 7]
    AtWA[:, 7, 7] = M[:, 6, 6] + M[:, 8, 8]
    AtWb = np.zeros((B, 8), f32)
    AtWb[:, 0:3] = M[:, 3, 0:3]
    AtWb[:, 3:6] = M[:, 4, 0:3]
    AtWb[:, 6] = -(M[:, 3, 5] + M[:, 4, 7])
    AtWb[:, 7] = -(M[:, 3, 6] + M[:, 4, 8])
    AtWA += REG * np.eye(8, dtype=f32)[None]
    h_id = np.array([1, 0, 0, 0, 1, 0, 0, 0], f32)
    AtWb += REG * h_id[None]

    try:
        h8 = np.linalg.solve(AtWA, AtWb[..., None])[..., 0].astype(f32)
    except np.linalg.LinAlgError:
        h8 = np.zeros((B, 8), f32)
        for b in range(B):
            try:
                h8[b] = np.linalg.solve(AtWA[b], AtWb[b])
            except np.linalg.LinAlgError:
                h8[b] = np.nan
    finite = np.all(np.isfinite(h8), axis=-1, keepdims=True)
    h8 = np.where(finite, h8, h_id[None])
    H_norm = np.concatenate([h8, np.ones((B, 1), f32)], axis=-1)
    H_norm = H_norm.reshape(B, 3, 3)

    T_src = np.zeros((B, 3, 3), f32)
    T_src[:, 0, 0] = sA
    T_src[:, 1, 1] = sA
    T_src[:, 0, 2] = -sA * cxA
    T_src[:, 1, 2] = -sA * cyA
    T_src[:, 2, 2] = 1.0
    s_dst = np.clip(sB, 1e-6, None)
    T_dst_inv = np.zeros((B, 3, 3), f32)
    T_dst_inv[:, 0, 0] = 1.0 / s_dst
    T_dst_inv[:, 1, 1] = 1.0 / s_dst
    T_dst_inv[:, 0, 2] = (sB * cxB) / s_dst
    T_dst_inv[:, 1, 2] = (sB * cyB) / s_dst
    T_dst_inv[:, 2, 2] = 1.0

    H = (T_dst_inv @ (H_norm @ T_src)).astype(f32)
    H = H / np.clip(np.abs(H[:, 2:3, 2:3]), 1e-8, None)
    h33 = H[:, 2:3, 2:3]
    sgn = np.sign(h33)
    sgn = np.where(sgn == 0, np.ones_like(sgn), sgn)
    H = H / (np.clip(np.abs(h33), 1e-8, None) * sgn)
    H_finite = np.all(np.isfinite(H), axis=(-2, -1))
    a33 = np.abs(H[:, 2, 2])
    valid = H_finite & (a33 > 1e-4) & (a33 < 1e4)
    eye = np.eye(3, dtype=f32)
    H = np.where(valid[:, None, None], H, eye[None])
    return H.astype(f32)


# revision 30
# speedup vs baseline: 1.3224x; 1.0599x over previous
"""AgriMatcher Trainium2 kernel: point-matching network + weighted-DLT homography.

Data-parallel over batch B=64 across 8 NeuronCores (8 images/core). The device
runs the network (fc-compression + LayerNorm + gelu, PointNet encoder, weight
head) and accumulates the per-image 9x9 weighted Gram matrix
M = sum_n w_n q_n q_n^T over Hartley-normalized point monomials q (host-built).
Host assembles AtWA/AtWb from M, solves 8x8, composes the 3x3 homographies.

Perf structure:
- fc1 and all five 128-wide layers run as fp8(E4M3) DoubleRow matmuls
  (2 fp8 rows/PE-cell = 2x-4x tensor throughput). Weights carry per-row pow2
  scales, undone by each evacuation's per-partition scale; layer biases ride a
  static ones-row in the DoubleRow zero-plane.
- Host precomputes |fA-fB| and fA*fB (fp8, pow2-scaled), the DLT q monomials,
  and a 3-way fp8 hi/mid/lo split of the positions for the encoder input.
- LayerNorm via PE transposes; variance by fused square+accumulate
  (scalar_tensor_tensor) on Vector; rstd (fast-invsqrt + Newton) on Vector;
  per-chunk rstd apply on GpSimd.
- Evacuations (PSUM->SBUF w/ relu+scale) balanced across Scalar and Vector.
"""

import numpy as np
import ml_dtypes

import concourse.bass as bass
import concourse.mybir as mybir
import concourse.tile as tile
from concourse import bacc, bass_utils
from concourse.masks import make_identity

F32 = mybir.dt.float32
BF16 = mybir.dt.bfloat16
FP8 = mybir.dt.float8e4
I32 = mybir.dt.int32
AF = mybir.ActivationFunctionType
OP = mybir.AluOpType
AX = mybir.AxisListType
PM = mybir.MatmulPerfMode

B, N, C = 64, 4096, 128
HID, COMP = 128, 32
NCORES = 8
BL = B // NCORES          # images per core
TILE = 1024               # points per tile
NT = N // TILE            # tiles per image (4)
NCH = TILE // 128         # 128-pt chunks per tile (8)
NTC = BL * NT             # tiles per core (32)
NC32 = N // 128           # 128-pt chunks per image (32)
EPS = 1e-5
REG = 1e-4
MAGIC = 0x5F3759DF

BF = ml_dtypes.bfloat16
E4 = ml_dtypes.float8_e4m3


def build():
    nc = bacc.Bacc("TRN2", target_bir_lowering=False, debug=False,
                   num_devices=NCORES)

    dm = nc.dram_tensor("dm", [BL, 128, 2, N], FP8, kind="ExternalInput").ap()
    posq = nc.dram_tensor("posq", [BL, 12, N], FP8, kind="ExternalInput").ap()
    zpl = nc.dram_tensor("zpl", [128, TILE], FP8, kind="ExternalInput").ap()
    # params
    w1q = nc.dram_tensor("w1q", [128, 2, 64], FP8, kind="ExternalInput").ap()
    sfc = nc.dram_tensor("sfc", [64, 1], F32, kind="ExternalInput").ap()
    b1c = nc.dram_tensor("b1c", [64, 1], F32, kind="ExternalInput").ap()
    gcol = nc.dram_tensor("gcol", [64, 1], F32, kind="ExternalInput").ap()
    bln = nc.dram_tensor("bln", [64, 1], F32, kind="ExternalInput").ap()
    we0 = nc.dram_tensor("we0", [76, 2, 128], FP8, kind="ExternalInput").ap()
    s0c = nc.dram_tensor("s0c", [128, 1], F32, kind="ExternalInput").ap()
    we1 = nc.dram_tensor("we1", [128, 2, 128], FP8, kind="ExternalInput").ap()
    s1c = nc.dram_tensor("s1c", [128, 1], F32, kind="ExternalInput").ap()
    we2 = nc.dram_tensor("we2", [128, 2, 128], FP8, kind="ExternalInput").ap()
    s2c = nc.dram_tensor("s2c", [128, 1], F32, kind="ExternalInput").ap()
    w0a = nc.dram_tensor("w0a", [128, 2, 128], FP8, kind="ExternalInput").ap()
    s3c = nc.dram_tensor("s3c", [128, 1], F32, kind="ExternalInput").ap()
    w0b = nc.dram_tensor("w0b", [128, 128], BF16, kind="ExternalInput").ap()
    bh0c = nc.dram_tensor("bh0c", [128, 1], F32, kind="ExternalInput").ap()
    wh1 = nc.dram_tensor("wh1", [128, 2, 64], FP8, kind="ExternalInput").ap()
    s4c = nc.dram_tensor("s4c", [64, 1], F32, kind="ExternalInput").ap()
    w2col = nc.dram_tensor("w2col", [64, 1], BF16, kind="ExternalInput").ap()

    out = nc.dram_tensor("out", [BL, NT, 33, 512], F32,
                         kind="ExternalOutput").ap()

    with tile.TileContext(nc) as tc:
        with (
            tc.tile_pool(name="const", bufs=1) as cp,
            tc.tile_pool(name="persist", bufs=1) as pp,
            tc.tile_pool(name="work", bufs=3) as wp,
            tc.tile_pool(name="feat", bufs=4) as fp,
            tc.tile_pool(name="ps", bufs=2, space="PSUM") as ps,
            tc.tile_pool(name="psb", bufs=2, space="PSUM") as psb,
        ):
            ident = cp.tile([128, 128], BF16)
            make_identity(nc, ident)

            def cload(ap_in, shape, dtype):
                t = cp.tile(shape, dtype, tag=ap_in.tensor.name)
                nc.sync.dma_start(out=t, in_=ap_in)
                return t

            w1q_t = cload(w1q, [128, 2, 64], FP8)
            sfc_t = cload(sfc, [64, 1], F32)
            b1c_t = cload(b1c, [64, 1], F32)
            gcol_t = cload(gcol, [64, 1], F32)
            bln_t = cload(bln, [64, 1], F32)
            we0_t = cload(we0, [76, 2, 128], FP8)
            s0c_t = cload(s0c, [128, 1], F32)
            we1_t = cload(we1, [128, 2, 128], FP8)
            s1c_t = cload(s1c, [128, 1], F32)
            we2_t = cload(we2, [128, 2, 128], FP8)
            s2c_t = cload(s2c, [128, 1], F32)
            w0a_t = cload(w0a, [128, 2, 128], FP8)
            s3c_t = cload(s3c, [128, 1], F32)
            w0b_t = cload(w0b, [128, 128], BF16)
            bh0c_t = cload(bh0c, [128, 1], F32)
            wh1_t = cload(wh1, [128, 2, 64], FP8)
            s4c_t = cload(s4c, [64, 1], F32)
            w2col_t = cload(w2col, [64, 1], BF16)

            # persistent state
            hc_all = pp.tile([128, NTC, NCH, 64], BF16)
            s2_all = pp.tile([128, NTC, NCH], F32)
            rstd_all = pp.tile([128, NTC * NCH], F32)
            rstd_bf = pp.tile([128, NTC * NCH], BF16)
            vp_all = pp.tile([128, NTC * NCH], F32)
            u_all = pp.tile([128, NTC * NCH], F32)
            gparts = pp.tile([128, BL, NT], F32)

            # fp8 activation tiles: [*, 2, TILE], plane 1 = zero pad with
            # ones at partitions 0/1 (DoubleRow bias rows). DMA'd once.
            hg_b = [pp.tile([76, 2, TILE], FP8, tag=f"hg{i}", name=f"hg{i}")
                    for i in range(2)]
            x1_b = [pp.tile([128, 2, TILE], FP8, tag=f"x1{i}", name=f"x1{i}")
                    for i in range(2)]
            x2_b = [pp.tile([128, 2, TILE], FP8, tag=f"x2{i}", name=f"x2{i}")
                    for i in range(2)]
            y0_b = [pp.tile([128, 2, TILE], FP8, tag=f"y0{i}", name=f"y0{i}")
                    for i in range(2)]
            # local: plane 0..3 = data tiles, plane 4 = zero pad
            loc_b = [pp.tile([128, NT + 1, TILE], FP8, tag=f"lc{i}",
                             name=f"lc{i}") for i in range(2)]
            y1_b = [pp.tile([64, TILE], BF16, tag=f"y1{i}", name=f"y1{i}")
                    for i in range(2)]

            for t in hg_b:
                nc.sync.dma_start(out=t[0:76, 1, :], in_=zpl[0:76, :])
            for t in x1_b + x2_b + y0_b:
                nc.sync.dma_start(out=t[:, 1, :], in_=zpl)
            for t in loc_b:
                nc.sync.dma_start(out=t[:, NT, :], in_=zpl)

            def dr_rhs(t, plane, zplane, sl):
                base = t[:, plane, sl]
                return bass.AP(tensor=base.tensor, offset=base.offset,
                               ap=[base.ap[0],
                                   [(zplane - plane) * TILE, 2],
                                   base.ap[-1]])

            # ---------------- phase 1: fc1 + LN stats ----------------
            def p1_load(st):
                img, ti = st["img"], st["ti"]
                p0 = ti * TILE
                dm_t = fp.tile([128, 2, TILE], FP8, tag="dm")
                nc.sync.dma_start(out=dm_t, in_=dm[img, :, :, p0:p0 + TILE])
                st["dm"] = dm_t

            def p1_fc1(st):
                h_ps = ps.tile([64, TILE], F32, tag="big")
                dm_t = st["dm"]
                for half in range(TILE // 512):
                    sl = slice(half * 512, half * 512 + 512)
                    nc.tensor.matmul(h_ps[:, sl], w1q_t, dm_t[:, :, sl],
                                     start=True, stop=True,
                                     perf_mode=PM.DoubleRow)
                st["h_ps"] = h_ps

            def p1_evac(st):
                h_sb = wp.tile([64, TILE], BF16, tag="h_sb")
                nc.scalar.activation(h_sb, st["h_ps"], AF.Identity,
                                     bias=b1c_t, scale=sfc_t)
                st["h_sb"] = h_sb

            def p1_fwdT(st):
                hp_ps = psb.tile([128, NCH, 64], BF16, tag="tp")
                h_sb = st["h_sb"]
                for j in range(NCH):
                    nc.tensor.transpose(hp_ps[:, j, :],
                                        h_sb[:, j * 128:(j + 1) * 128],
                                        ident[:64, :64])
                st["hp_ps"] = hp_ps

            def p1_sq(st):
                t = st["t"]
                hp = st["hp_ps"]
                hcf = hc_all[:, t].rearrange("p a b -> p (a b)")
                nc.vector.tensor_copy(hcf, hp.rearrange("p a b -> p (a b)"))
                sqd = wp.tile([128, NCH, 64], BF16, tag="sqd")
                nc.vector.tensor_mul(
                    sqd.rearrange("p a b -> p (a b)"), hcf, hcf)
                nc.vector.reduce_sum(out=s2_all[:, t], in_=sqd, axis=AX.X)

            P1_STAGES = [p1_fc1, p1_evac, p1_fwdT, p1_sq]

            def newton_all():
                s2f = s2_all.rearrange("p a b -> p (a b)")
                vp, yv, u_t = vp_all, rstd_all, u_all
                nc.vector.tensor_scalar(vp, s2f, 1.0 / 64.0, EPS,
                                        op0=OP.mult, op1=OP.add)
                nc.vector.tensor_scalar(yv.bitcast(I32), vp.bitcast(I32), 1,
                                        None, op0=OP.arith_shift_right)
                nc.vector.tensor_scalar(yv.bitcast(I32), yv.bitcast(I32),
                                        0xFFFFFFFF, None, op0=OP.bitwise_xor)
                nc.vector.tensor_scalar(yv.bitcast(I32), yv.bitcast(I32),
                                        MAGIC + 1, None, op0=OP.add)
                for _ in range(3):
                    nc.vector.tensor_mul(u_t, yv, yv)
                    nc.vector.tensor_mul(u_t, u_t, vp)
                    nc.vector.tensor_scalar(u_t, u_t, -0.5, 1.5,
                                            op0=OP.mult, op1=OP.add)
                    nc.vector.tensor_mul(yv, yv, u_t)
                nc.vector.tensor_copy(rstd_bf, rstd_all)

            # ---------------- phase 2 stages ----------------
            def e_apply(st):
                t = st["t"]
                hcn = wp.tile([128, NCH, 64], BF16, tag="hcn")
                rb = rstd_bf[:, t * NCH:(t + 1) * NCH]
                rb_bc = bass.AP(tensor=rb.tensor, offset=rb.offset,
                                ap=[rb.ap[0], rb.ap[-1], [0, 64]])
                nc.gpsimd.tensor_tensor(out=hcn, in0=hc_all[:, t],
                                        in1=rb_bc, op=OP.mult)
                st["hcn"] = hcn
                # prefetch positions into the hg buffer
                img, ti = st["img"], st["ti"]
                p0 = ti * TILE
                hg_t = st["hg"]
                nc.sync.dma_start(out=hg_t[64:76, 0, :],
                                  in_=posq[img, :, p0:p0 + TILE])

            def e_bwdT(st):
                ycm = psb.tile([64, TILE], BF16, tag="tp")
                hcn = st["hcn"]
                for j in range(NCH):
                    nc.tensor.transpose(ycm[:, j * 128:(j + 1) * 128],
                                        hcn[:, j], ident)
                st["ycm"] = ycm

            def e_gelu(st):
                hg_t = st["hg"]
                nc.scalar.activation(hg_t[0:64, 0, :], st["ycm"], AF.Gelu,
                                     bias=bln_t, scale=gcol_t)

            def e_enc0(st):
                e0 = ps.tile([128, TILE], F32, tag="big")
                hg_t = st["hg"]
                for half in range(TILE // 512):
                    sl = slice(half * 512, half * 512 + 512)
                    nc.tensor.matmul(e0[:, sl], we0_t,
                                     dr_rhs(hg_t, 0, 1, sl),
                                     start=True, stop=True,
                                     perf_mode=PM.DoubleRow)
                st["e0"] = e0

            def e_x1(st):
                x1_t = st["x1"]
                nc.scalar.activation(x1_t[:, 0, :], st["e0"], AF.Relu,
                                     scale=s0c_t)

            def e_enc1(st):
                e1 = ps.tile([128, TILE], F32, tag="big")
                x1_t = st["x1"]
                for half in range(TILE // 512):
                    sl = slice(half * 512, half * 512 + 512)
                    nc.tensor.matmul(e1[:, sl], we1_t, dr_rhs(x1_t, 0, 1, sl),
                                     start=True, stop=True,
                                     perf_mode=PM.DoubleRow)
                st["e1"] = e1

            def e_x2(st):
                x2_t = st["x2"]
                nc.vector.tensor_scalar(x2_t[:, 0, :], st["e1"], s1c_t, 0.0,
                                        op0=OP.mult, op1=OP.max)

            def e_enc2(st):
                e2 = ps.tile([128, TILE], F32, tag="big")
                x2_t = st["x2"]
                for half in range(TILE // 512):
                    sl = slice(half * 512, half * 512 + 512)
                    nc.tensor.matmul(e2[:, sl], we2_t, dr_rhs(x2_t, 0, 1, sl),
                                     start=True, stop=True,
                                     perf_mode=PM.DoubleRow)
                st["e2"] = e2

            def e_loc(st):
                img, ti = st["img"], st["ti"]
                loc_t = st["loc"]
                e2 = st["e2"]
                # split the evacuation S/V for balance
                nc.scalar.activation(loc_t[:, ti, 0:512], e2[:, 0:512],
                                     AF.Relu, scale=s2c_t)
                nc.vector.tensor_scalar(loc_t[:, ti, 512:TILE],
                                        e2[:, 512:TILE], s2c_t, 0.0,
                                        op0=OP.mult, op1=OP.max)
                nc.vector.reduce_max(out=gparts[:, img, ti:ti + 1],
                                     in_=e2, axis=AX.X)

            E_STAGES = [e_apply, e_bwdT, e_gelu, e_enc0, e_x1, e_enc1, e_x2,
                        e_enc2, e_loc]

            def glob_s0(sh):
                img = sh["img"]
                graw = wp.tile([128, 1], F32, tag="graw")
                nc.vector.reduce_max(out=graw, in_=gparts[:, img], axis=AX.X)
                gbf = wp.tile([128, 1], BF16, tag="gbf")
                nc.scalar.activation(gbf, graw, AF.Relu, scale=s2c_t)
                gmm = psb.tile([128, NCH], F32, tag="wz", name="gmm")
                nc.tensor.matmul(gmm[:, 0:1], w0b_t, gbf,
                                 start=True, stop=True)
                b0h = wp.tile([128, 1], F32, tag="b0h")
                nc.vector.tensor_scalar(b0h, gmm[:, 0:1], bh0c_t, None,
                                        op0=OP.add)
                sh["b0h"] = b0h

            def h_h0(st):
                h0 = ps.tile([128, TILE], F32, tag="big")
                loc_t = st["loc"]
                ti = st["ti"]
                for half in range(TILE // 512):
                    sl = slice(half * 512, half * 512 + 512)
                    nc.tensor.matmul(h0[:, sl], w0a_t,
                                     dr_rhs(loc_t, ti, NT, sl),
                                     start=True, stop=True,
                                     perf_mode=PM.DoubleRow)
                st["h0"] = h0

            def h_y0(st):
                y0_t = st["y0"]
                nc.scalar.activation(y0_t[:, 0, :], st["h0"], AF.Relu,
                                     bias=st["sh"]["b0h"], scale=s3c_t)

            def h_h1(st):
                h1 = ps.tile([64, TILE], F32, tag="big")
                y0_t = st["y0"]
                for half in range(TILE // 512):
                    sl = slice(half * 512, half * 512 + 512)
                    nc.tensor.matmul(h1[:, sl], wh1_t, dr_rhs(y0_t, 0, 1, sl),
                                     start=True, stop=True,
                                     perf_mode=PM.DoubleRow)
                st["h1"] = h1

            def h_y1(st):
                y1_t = st["y1"]
                nc.vector.tensor_scalar(y1_t, st["h1"], s4c_t, 0.0,
                                        op0=OP.mult, op1=OP.max)

            def h_dots(st):
                # wz[point] = w2 . y1 as two [1, 512]-out matmuls per tile,
                # packed at partitions 0/32 of one PSUM bank; raw wz ships
                # to the host (which applies the sigmoid).
                img, ti = st["img"], st["ti"]
                wz_ps = psb.tile([33, 512], F32, tag="wz", name="wz_ps")
                y1_t = st["y1"]
                for h in range(2):
                    nc.tensor.matmul(wz_ps[32 * h:32 * h + 1, :], w2col_t,
                                     y1_t[:, h * 512:(h + 1) * 512],
                                     start=True, stop=True)
                wz_sb = wp.tile([33, 512], F32, tag="wzs")
                nc.scalar.copy(wz_sb, wz_ps)
                nc.sync.dma_start(out=out[img, ti], in_=wz_sb)

            H_STAGES = [h_h0, h_y0, h_h1, h_y1, h_dots]

            # ---------------- schedule ----------------
            def run_window(units, W=2):
                active = []
                idx = 0
                while idx < len(units) or active:
                    while len(active) < W and idx < len(units):
                        stages, st = units[idx]
                        active.append([stages, st, 0])
                        idx += 1
                    for u in list(active):
                        stages, st, k = u
                        stages[k](st)
                        u[2] += 1
                        if u[2] >= len(stages):
                            active.remove(u)

            # phase 1 with deep DMA prefetch
            p1_sts = [{"img": img, "ti": ti, "t": img * NT + ti}
                      for img in range(BL) for ti in range(NT)]
            for st in p1_sts[:4]:
                p1_load(st)
            for i in range(0, len(p1_sts), 2):
                pair = p1_sts[i:i + 2]
                for st in p1_sts[i + 4:i + 6]:
                    p1_load(st)
                for stg in P1_STAGES:
                    for st in pair:
                        stg(st)

            newton_all()

            # software-pipeline across images: enc(img) runs alongside
            # head(img-1) so the per-image glob barrier never drains the
            # window.
            shs = [{"img": img} for img in range(BL)]
            e_units = {img: [] for img in range(BL)}
            h_units = {img: [] for img in range(BL)}
            for img in range(BL):
                loc_t = loc_b[img % 2]
                for ti in range(NT):
                    k = (img * NT + ti) % 2
                    st = {"img": img, "ti": ti, "t": img * NT + ti,
                          "sh": shs[img], "hg": hg_b[k], "x1": x1_b[k],
                          "x2": x2_b[k], "y0": y0_b[k], "y1": y1_b[k],
                          "loc": loc_t}
                    e_units[img].append((E_STAGES, st))
                    h_units[img].append((H_STAGES, st))
            units = []
            for img in range(BL + 1):
                for ti in range(NT):
                    if img < BL:
                        units.append(e_units[img][ti])
                    if img >= 1:
                        units.append(h_units[img - 1][ti])
                if img < BL:
                    units.append(([glob_s0], shs[img]))
            run_window(units, W=2)

    nc.compile()
    return nc


_CACHE = {}


def _get_nc():
    if "nc" not in _CACHE:
        _CACHE["nc"] = build()
    return _CACHE["nc"]


def _hartley(pts):
    pts = pts.astype(np.float32)
    centroid = pts.mean(axis=1, keepdims=True)
    pc = pts - centroid
    dist = np.sqrt(np.clip((pc ** 2).sum(-1), 0.0, None))
    mean_dist = dist.mean(axis=1, keepdims=True)
    scale = np.float32(np.sqrt(2.0)) / np.clip(mean_dist, 0.001, None)
    scale = np.where(mean_dist < 0.001, np.ones_like(scale), scale)
    pts_norm = pc * scale[..., None]
    return (pts_norm.astype(np.float32), scale[:, 0].astype(np.float32),
            centroid[:, 0, 0].astype(np.float32),
            centroid[:, 0, 1].astype(np.float32))


def _pow2(x):
    """Largest power of two <= x (elementwise, safe)."""
    x = np.maximum(np.asarray(x, np.float64), 1e-30)
    return np.exp2(np.floor(np.log2(x))).astype(np.float32)


def _rowquant(Wrow_mats, target=120.0):
    """Per-row pow2 scale k for a list of [out, in_i] f32 matrices sharing
    rows. Returns (list of fp8 matrices scaled by k, inv_scale [out] f32)."""
    mx = np.zeros(Wrow_mats[0].shape[0], np.float64)
    for M in Wrow_mats:
        if M.size:
            mx = np.maximum(mx, np.abs(M).max(axis=1))
    k = _pow2(target / np.maximum(mx, 1e-30))
    k = np.clip(k, 2.0 ** -30, 2.0 ** 30).astype(np.float32)
    q = [np.clip(M * k[:, None], -240, 240).astype(E4) for M in Wrow_mats]
    return q, (1.0 / k).astype(np.float32)


def _fp8_bias_rows(b, k):
    """bias*k split into fp8 hi+lo rows."""
    z = np.clip(b * k, -240 * 1.99, 240 * 1.99).astype(np.float64)
    hi = np.clip(z, -240, 240).astype(E4)
    lo = np.clip(z - hi.astype(np.float64), -240, 240).astype(E4)
    return hi, lo


def kernel(pos_A, pos_B, feat_A, feat_B,
           fc_w1, fc_b1, fc_ln_g, fc_ln_b, fc_w2, fc_b2,
           enc_w0, enc_g0, enc_b0, enc_w1, enc_g1, enc_b1,
           enc_w2, enc_g2, enc_b2,
           head_w0, head_g0, head_b0, head_w1, head_g1, head_b1,
           head_w2, head_b2):
    f32 = np.float32
    pos_A = np.asarray(pos_A, f32)
    pos_B = np.asarray(pos_B, f32)
    fA = np.asarray(feat_A, f32)
    fB = np.asarray(feat_B, f32)

    # ---- folded f32 weights ----
    bnsc = f32(1.0 / np.sqrt(1.0 + EPS))
    w1c = (fc_w1 - fc_w1.mean(axis=0, keepdims=True)).astype(f32)
    b1cv = (fc_b1 - fc_b1.mean()).astype(f32)
    enc_w0s = (enc_w0 * (enc_g0 * bnsc)[:, None]).astype(f32)
    enc_w1s = (enc_w1 * (enc_g1 * bnsc)[:, None]).astype(f32)
    enc_w2s = (enc_w2 * (enc_g2 * bnsc)[:, None]).astype(f32)
    head_w0s = (head_w0 * (head_g0 * bnsc)[:, None]).astype(f32)
    head_w1s = (head_w1 * (head_g1 * bnsc)[:, None]).astype(f32)
    wfold = (enc_w0s[:, 4:36] @ fc_w2).astype(f32)          # [128, 64]
    benc0 = (enc_b0 + enc_w0s[:, 4:36] @ fc_b2).astype(f32)
    wpos = enc_w0s[:, 0:4].astype(f32)                      # [128, 4]
    w0a_f = head_w0s[:, 0:128]
    w0b_f = head_w0s[:, 128:256]

    # ---- input featurization ----
    d_f = np.abs(fA - fB)                                   # [B, N, C]
    m_f = fA * fB
    ad = float(_pow2(200.0 / max(d_f.max(), 1e-9)))
    am = float(_pow2(200.0 / max(np.abs(m_f).max(), 1e-9)))

    # ---- calibration on two images (f32 forward) ----
    def fwd(img):
        di, mi = d_f[img], m_f[img]
        h = np.concatenate([di, mi], -1) @ w1c.T + b1cv
        var = (h * h).mean(-1, keepdims=True)
        hn = h / np.sqrt(var + EPS) * fc_ln_g + fc_ln_b
        from scipy.special import erf
        hg = hn * 0.5 * (1.0 + erf(hn / np.sqrt(2.0)))
        e0 = hg @ wfold.T + np.concatenate([pos_A[img], pos_B[img]],
                                           -1) @ wpos.T + benc0
        x1 = np.maximum(e0, 0)
        x2 = np.maximum(x1 @ enc_w1s.T + enc_b1, 0)
        loc = np.maximum(x2 @ enc_w2s.T + enc_b2, 0)
        glob = loc.max(axis=0)
        b0h = head_b0 + w0b_f @ glob
        y0 = np.maximum(loc @ w0a_f.T + b0h, 0)
        y1 = np.maximum(y0 @ head_w1s.T + head_b1, 0)
        return (x1.max(0), x2.max(0), loc.max(0), y0.max(0), y1.max(0))

    mxs = [np.maximum(a, b) for a, b in zip(fwd(0), fwd(B // 2))]
    tgt = 48.0
    a1 = _pow2(tgt / np.maximum(mxs[0].max(), 1e-9))
    a2 = _pow2(tgt / np.maximum(mxs[1].max(), 1e-9))
    al = _pow2(tgt / np.maximum(mxs[2].max(), 1e-9))
    a3 = _pow2(tgt / np.maximum(mxs[3].max(), 1e-9))

    # ---- quantized params ----
    (w1d_q, w1m_q), inv1 = _rowquant([w1c[:, 0:128] / ad,
                                      w1c[:, 128:256] / am])
    w1q_h = np.zeros((128, 2, 64), E4)
    w1q_h[:, 0, :] = w1d_q.T
    w1q_h[:, 1, :] = w1m_q.T

    (we0_q,), inv0 = _rowquant(
        [np.concatenate([wfold, wpos * 4, wpos * 4, wpos * 4], axis=1)])
    k0 = 1.0 / inv0
    b0hi, b0lo = _fp8_bias_rows(benc0, k0)
    we0_h = np.zeros((76, 2, 128), E4)
    we0_h[:, 0, :] = we0_q.T
    we0_h[0, 1, :] = b0hi
    we0_h[1, 1, :] = b0lo

    (we1_q,), invs1 = _rowquant([enc_w1s / a1])
    b1hi, b1lo = _fp8_bias_rows(enc_b1, 1.0 / invs1)
    we1_h = np.zeros((128, 2, 128), E4)
    we1_h[:, 0, :] = we1_q.T
    we1_h[0, 1, :] = b1hi
    we1_h[1, 1, :] = b1lo

    (we2_q,), invs2 = _rowquant([enc_w2s / a2])
    b2hi, b2lo = _fp8_bias_rows(enc_b2, 1.0 / invs2)
    we2_h = np.zeros((128, 2, 128), E4)
    we2_h[:, 0, :] = we2_q.T
    we2_h[0, 1, :] = b2hi
    we2_h[1, 1, :] = b2lo

    (w0a_q,), invs3 = _rowquant([w0a_f / al])
    w0a_h = np.zeros((128, 2, 128), E4)
    w0a_h[:, 0, :] = w0a_q.T

    (wh1_q,), invs4 = _rowquant([head_w1s / a3])
    bh1hi, bh1lo = _fp8_bias_rows(head_b1, 1.0 / invs4)
    wh1_h = np.zeros((128, 2, 64), E4)
    wh1_h[:, 0, :] = wh1_q.T
    wh1_h[0, 1, :] = bh1hi
    wh1_h[1, 1, :] = bh1lo

    params = {
        "w1q": w1q_h,
        "sfc": inv1.reshape(64, 1),
        "b1c": b1cv.reshape(64, 1),
        "gcol": fc_ln_g.astype(f32).reshape(64, 1),
        "bln": fc_ln_b.astype(f32).reshape(64, 1),
        "we0": we0_h, "s0c": (inv0 * a1).reshape(128, 1).astype(f32),
        "we1": we1_h, "s1c": (invs1 * a2).reshape(128, 1).astype(f32),
        "we2": we2_h, "s2c": (invs2 * al).reshape(128, 1).astype(f32),
        "w0a": w0a_h, "s3c": (invs3 * a3).reshape(128, 1).astype(f32),
        "w0b": (w0b_f * (a3 / al)).T.astype(BF),
        "bh0c": (head_b0 * a3).reshape(128, 1).astype(f32),
        "wh1": wh1_h, "s4c": invs4.reshape(64, 1).astype(f32),
        "w2col": head_w2.reshape(64, 1).astype(BF),
    }

    # zero/ones plane: partitions 0/1 = 1.0, rest 0
    zp = np.zeros((128, TILE), f32)
    zp[0:2] = 1.0
    params["zpl"] = zp.astype(E4)

    # ---- per-core data tensors ----
    # d/m packed [B, 128, 2, N] fp8
    dmh = np.empty((B, C, 2, N), E4)
    dmh[:, :, 0, :] = np.clip(d_f * ad, 0, 240).transpose(0, 2, 1).astype(E4)
    dmh[:, :, 1, :] = np.clip(m_f * am, -240, 240).transpose(0, 2, 1).astype(E4)
    # but the weights were built for d_true = d_stored/ad with stored=d*ad:
    # inv1 built from w/ad so PSUM = k*(w.d_true*ad/ad)... handled above.

    # positions hi/mid/lo fp8, scaled by 1/4
    psc = np.concatenate([pos_A, pos_B], -1).transpose(0, 2, 1) * 0.25
    hi = np.clip(psc, -240, 240).astype(E4)
    r1 = psc - hi.astype(f32)
    mid = np.clip(r1, -240, 240).astype(E4)
    r2 = r1 - mid.astype(f32)
    lo = np.clip(r2, -240, 240).astype(E4)
    posq_h = np.concatenate([hi, mid, lo], axis=1)          # [B, 12, N]

    # q monomials (Hartley-normalized)
    srcn, sA, cxA, cyA = _hartley(pos_A)
    dstn, sB, cxB, cyB = _hartley(pos_B)
    sx, sy = srcn[..., 0], srcn[..., 1]
    dx, dy = dstn[..., 0], dstn[..., 1]
    one = np.ones_like(sx)
    q9 = np.stack([sx, sy, one, dx, dy, dx * sx, dx * sy, dy * sx, dy * sy],
                  axis=-1)                                   # [B, N, 9]

    in_maps = []
    for i in range(NCORES):
        sl = slice(i * BL, (i + 1) * BL)
        mcore = {"dm": np.ascontiguousarray(dmh[sl]),
                 "posq": np.ascontiguousarray(posq_h[sl])}
        mcore.update(params)
        in_maps.append(mcore)

    nc = _get_nc()
    res = bass_utils.run_bass_kernel_spmd(nc, in_maps,
                                          core_ids=list(range(NCORES)))
    wzr = np.concatenate([res.results[i]["out"] for i in range(NCORES)],
                         axis=0)                  # [B, NT, 33, 512]
    wz = wzr[:, :, [0, 32], :].reshape(B, N).astype(np.float64)
    w = 1.0 / (1.0 + np.exp(-(wz + float(head_b2[0]))))
    w = w.astype(f32)
    qw = q9 * w[:, :, None]
    M = np.einsum('bnk,bnl->bkl', qw, q9, optimize=True).astype(f32)

    # ---- host post: assemble AtWA/AtWb, solve, compose ----
    u3 = [0, 1, 2]
    AtWA = np.zeros((B, 8, 8), f32)
    AtWA[:, 0:3, 0:3] = M[:, 0:3, 0:3]
    AtWA[:, 3:6, 3:6] = M[:, 0:3, 0:3]
    AtWA[:, 0:3, 6] = -M[:, u3, 5]
    AtWA[:, 0:3, 7] = -M[:, u3, 6]
    AtWA[:, 3:6, 6] = -M[:, u3, 7]
    AtWA[:, 3:6, 7] = -M[:, u3, 8]
    AtWA[:, 6, 0:3] = -M[:, u3, 5]
    AtWA[:, 7, 0:3] = -M[:, u3, 6]
    AtWA[:, 6, 3:6] = -M[:, u3, 7]
    AtWA[:, 7, 3:6] = -M[:, u3, 8]
    AtWA[:, 6, 6] = M[:, 5, 5] + M[:, 7, 7]
    AtWA[:, 6, 7] = M[:, 5, 6] + M[:, 7, 8]
    AtWA[:, 7, 6] = M[:, 6, 5] + M[:, 8, 7]
    AtWA[:, 7, 7] = M[:, 6, 6] + M[:, 8, 8]
    AtWb = np.zeros((B, 8), f32)
    AtWb[:, 0:3] = M[:, 3, 0:3]
    AtWb[:, 3:6] = M[:, 4, 0:3]
    AtWb[:, 6] = -(M[:, 3, 5] + M[:, 4, 7])
    AtWb[:, 7] = -(M[:, 3, 6] + M[:, 4, 8])
    AtWA += REG * np.eye(8, dtype=f32)[None]
    h_id = np.array([1, 0, 0, 0, 1, 0, 0, 0], f32)
    AtWb += REG * h_id[None]

    try:
        h8 = np.linalg.solve(AtWA, AtWb[..., None])[..., 0].astype(f32)
    except np.linalg.LinAlgError:
        h8 = np.zeros((B, 8), f32)
        for b in range(B):
            try:
                h8[b] = np.linalg.solve(AtWA[b], AtWb[b])
            except np.linalg.LinAlgError:
                h8[b] = np.nan
    finite = np.all(np.isfinite(h8), axis=-1, keepdims=True)
    h8 = np.where(finite, h8, h_id[None])
    H_norm = np.concatenate([h8, np.ones((B, 1), f32)], axis=-1)
    H_norm = H_norm.reshape(B, 3, 3)

    T_src = np.zeros((B, 3, 3), f32)
    T_src[:, 0, 0] = sA
    T_src[:, 1, 1] = sA
    T_src[:, 0, 2] = -sA * cxA
    T_src[:, 1, 2] = -sA * cyA
    T_src[:, 2, 2] = 1.0
    s_dst = np.clip(sB, 1e-6, None)
    T_dst_inv = np.zeros((B, 3, 3), f32)
    T_dst_inv[:, 0, 0] = 1.0 / s_dst
    T_dst_inv[:, 1, 1] = 1.0 / s_dst
    T_dst_inv[:, 0, 2] = (sB * cxB) / s_dst
    T_dst_inv[:, 1, 2] = (sB * cyB) / s_dst
    T_dst_inv[:, 2, 2] = 1.0

    H = (T_dst_inv @ (H_norm @ T_src)).astype(f32)
    H = H / np.clip(np.abs(H[:, 2:3, 2:3]), 1e-8, None)
    h33 = H[:, 2:3, 2:3]
    sgn = np.sign(h33)
    sgn = np.where(sgn == 0, np.ones_like(sgn), sgn)
    H = H / (np.clip(np.abs(h33), 1e-8, None) * sgn)
    H_finite = np.all(np.isfinite(H), axis=(-2, -1))
    a33 = np.abs(H[:, 2, 2])
    valid = H_finite & (a33 > 1e-4) & (a33 < 1e4)
    eye = np.eye(3, dtype=f32)
    H = np.where(valid[:, None, None], H, eye[None])
    return H.astype(f32)


# revision 36
# speedup vs baseline: 1.3374x; 1.0114x over previous
"""AgriMatcher Trainium2 kernel: point-matching network + weighted-DLT homography.

Data-parallel over batch B=64 across 8 NeuronCores (8 images/core). The device
runs the network (fc-compression + LayerNorm + gelu, PointNet encoder, weight
head) and accumulates the per-image 9x9 weighted Gram matrix
M = sum_n w_n q_n q_n^T over Hartley-normalized point monomials q (host-built).
Host assembles AtWA/AtWb from M, solves 8x8, composes the 3x3 homographies.

Perf structure:
- fc1 and all five 128-wide layers run as fp8(E4M3) DoubleRow matmuls
  (2 fp8 rows/PE-cell = 2x-4x tensor throughput). Weights carry per-row pow2
  scales, undone by each evacuation's per-partition scale; layer biases ride a
  static ones-row in the DoubleRow zero-plane.
- Host precomputes |fA-fB| and fA*fB (fp8, pow2-scaled), the DLT q monomials,
  and a 3-way fp8 hi/mid/lo split of the positions for the encoder input.
- LayerNorm via PE transposes; variance by fused square+accumulate
  (scalar_tensor_tensor) on Vector; rstd (fast-invsqrt + Newton) on Vector;
  per-chunk rstd apply on GpSimd.
- Evacuations (PSUM->SBUF w/ relu+scale) balanced across Scalar and Vector.
"""

import numpy as np
import ml_dtypes

import concourse.bass as bass
import concourse.mybir as mybir
import concourse.tile as tile
from concourse import bacc, bass_utils
from concourse.masks import make_identity

F32 = mybir.dt.float32
BF16 = mybir.dt.bfloat16
FP8 = mybir.dt.float8e4
I32 = mybir.dt.int32
AF = mybir.ActivationFunctionType
OP = mybir.AluOpType
AX = mybir.AxisListType
PM = mybir.MatmulPerfMode

B, N, C = 64, 4096, 128
HID, COMP = 128, 32
NCORES = 8
BL = B // NCORES          # images per core
TILE = 1024               # points per tile
NT = N // TILE            # tiles per image (4)
NCH = TILE // 128         # 128-pt chunks per tile (8)
NTC = BL * NT             # tiles per core (32)
NC32 = N // 128           # 128-pt chunks per image (32)
EPS = 1e-5
REG = 1e-4
MAGIC = 0x5F3759DF

BF = ml_dtypes.bfloat16
E4 = ml_dtypes.float8_e4m3


def build():
    nc = bacc.Bacc("TRN2", target_bir_lowering=False, debug=False,
                   num_devices=NCORES)

    dm = nc.dram_tensor("dm", [BL, 128, 2, N], FP8, kind="ExternalInput").ap()
    posq = nc.dram_tensor("posq", [BL, 12, N], FP8, kind="ExternalInput").ap()
    zpl = nc.dram_tensor("zpl", [128, TILE], FP8, kind="ExternalInput").ap()
    # params
    w1q = nc.dram_tensor("w1q", [128, 2, 64], FP8, kind="ExternalInput").ap()
    sfc = nc.dram_tensor("sfc", [64, 1], F32, kind="ExternalInput").ap()
    b1c = nc.dram_tensor("b1c", [64, 1], F32, kind="ExternalInput").ap()
    gcol = nc.dram_tensor("gcol", [64, 1], F32, kind="ExternalInput").ap()
    bln = nc.dram_tensor("bln", [64, 1], F32, kind="ExternalInput").ap()
    we0 = nc.dram_tensor("we0", [76, 2, 128], FP8, kind="ExternalInput").ap()
    s0c = nc.dram_tensor("s0c", [128, 1], F32, kind="ExternalInput").ap()
    we1 = nc.dram_tensor("we1", [128, 2, 128], FP8, kind="ExternalInput").ap()
    s1c = nc.dram_tensor("s1c", [128, 1], F32, kind="ExternalInput").ap()
    we2 = nc.dram_tensor("we2", [128, 2, 128], FP8, kind="ExternalInput").ap()
    s2c = nc.dram_tensor("s2c", [128, 1], F32, kind="ExternalInput").ap()
    w0a = nc.dram_tensor("w0a", [128, 2, 128], FP8, kind="ExternalInput").ap()
    s3c = nc.dram_tensor("s3c", [128, 1], F32, kind="ExternalInput").ap()
    w0b = nc.dram_tensor("w0b", [128, 128], BF16, kind="ExternalInput").ap()
    bh0c = nc.dram_tensor("bh0c", [128, 1], F32, kind="ExternalInput").ap()
    wh1 = nc.dram_tensor("wh1", [128, 2, 64], FP8, kind="ExternalInput").ap()
    w2r = nc.dram_tensor("w2r", [128, 64], BF16, kind="ExternalInput").ap()

    out = nc.dram_tensor("out", [BL, NT, 128, NCH], F32,
                         kind="ExternalOutput").ap()

    with tile.TileContext(nc) as tc:
        with (
            tc.tile_pool(name="const", bufs=1) as cp,
            tc.tile_pool(name="persist", bufs=1) as pp,
            tc.tile_pool(name="work", bufs=3) as wp,
            tc.tile_pool(name="feat", bufs=4) as fp,
            tc.tile_pool(name="ps", bufs=2, space="PSUM") as ps,
            tc.tile_pool(name="psb", bufs=2, space="PSUM") as psb,
        ):
            ident = cp.tile([128, 128], BF16)
            make_identity(nc, ident)

            def cload(ap_in, shape, dtype):
                t = cp.tile(shape, dtype, tag=ap_in.tensor.name)
                nc.sync.dma_start(out=t, in_=ap_in)
                return t

            w1q_t = cload(w1q, [128, 2, 64], FP8)
            sfc_t = cload(sfc, [64, 1], F32)
            b1c_t = cload(b1c, [64, 1], F32)
            gcol_t = cload(gcol, [64, 1], F32)
            bln_t = cload(bln, [64, 1], F32)
            we0_t = cload(we0, [76, 2, 128], FP8)
            s0c_t = cload(s0c, [128, 1], F32)
            we1_t = cload(we1, [128, 2, 128], FP8)
            s1c_t = cload(s1c, [128, 1], F32)
            we2_t = cload(we2, [128, 2, 128], FP8)
            s2c_t = cload(s2c, [128, 1], F32)
            w0a_t = cload(w0a, [128, 2, 128], FP8)
            s3c_t = cload(s3c, [128, 1], F32)
            w0b_t = cload(w0b, [128, 128], BF16)
            bh0c_t = cload(bh0c, [128, 1], F32)
            wh1_t = cload(wh1, [128, 2, 64], FP8)
            w2r_t = cload(w2r, [128, 64], BF16)

            # persistent state
            hc_all = pp.tile([128, NTC, NCH, 64], BF16)
            s2_all = pp.tile([128, NTC, NCH], F32)
            rstd_all = pp.tile([128, NTC * NCH], F32)
            rstd_bf = pp.tile([128, NTC * NCH], BF16)
            vp_all = pp.tile([128, NTC * NCH], F32)
            u_all = pp.tile([128, NTC * NCH], F32)
            gparts = pp.tile([128, BL, NT], F32)

            # fp8 activation tiles: [*, 2, TILE], plane 1 = zero pad with
            # ones at partitions 0/1 (DoubleRow bias rows). DMA'd once.
            hg_b = [pp.tile([76, 2, TILE], FP8, tag=f"hg{i}", name=f"hg{i}")
                    for i in range(2)]
            x1_b = [pp.tile([128, 2, TILE], FP8, tag=f"x1{i}", name=f"x1{i}")
                    for i in range(2)]
            x2_b = [pp.tile([128, 2, TILE], FP8, tag=f"x2{i}", name=f"x2{i}")
                    for i in range(2)]
            y0_b = [pp.tile([128, 2, TILE], FP8, tag=f"y0{i}", name=f"y0{i}")
                    for i in range(2)]
            # local: plane 0..3 = data tiles, plane 4 = zero pad
            loc_b = [pp.tile([128, NT + 1, TILE], FP8, tag=f"lc{i}",
                             name=f"lc{i}") for i in range(2)]
            y1_b = [pp.tile([128, NCH, 64], BF16, tag=f"y1{i}", name=f"y1{i}")
                    for i in range(2)]

            for t in hg_b:
                nc.sync.dma_start(out=t[0:76, 1, :], in_=zpl[0:76, :])
            for t in x1_b + x2_b + y0_b:
                nc.sync.dma_start(out=t[:, 1, :], in_=zpl)
            for t in loc_b:
                nc.sync.dma_start(out=t[:, NT, :], in_=zpl)

            def dr_rhs(t, plane, zplane, sl):
                base = t[:, plane, sl]
                return bass.AP(tensor=base.tensor, offset=base.offset,
                               ap=[base.ap[0],
                                   [(zplane - plane) * TILE, 2],
                                   base.ap[-1]])

            # ---------------- phase 1: fc1 + LN stats ----------------
            def p1_load(st):
                img, ti = st["img"], st["ti"]
                p0 = ti * TILE
                dm_t = fp.tile([128, 2, TILE], FP8, tag="dm")
                nc.sync.dma_start(out=dm_t, in_=dm[img, :, :, p0:p0 + TILE])
                st["dm"] = dm_t

            def p1_fc1(st):
                h_ps = ps.tile([64, TILE], F32, tag="big")
                dm_t = st["dm"]
                for half in range(TILE // 512):
                    sl = slice(half * 512, half * 512 + 512)
                    nc.tensor.matmul(h_ps[:, sl], w1q_t, dm_t[:, :, sl],
                                     start=True, stop=True,
                                     perf_mode=PM.DoubleRow)
                st["h_ps"] = h_ps

            def p1_evac(st):
                h_sb = wp.tile([64, TILE], BF16, tag="h_sb")
                nc.scalar.activation(h_sb, st["h_ps"], AF.Identity,
                                     bias=b1c_t, scale=sfc_t)
                st["h_sb"] = h_sb

            def p1_fwdT(st):
                hp_ps = psb.tile([128, NCH, 64], BF16, tag="tp")
                h_sb = st["h_sb"]
                for j in range(NCH):
                    nc.tensor.transpose(hp_ps[:, j, :],
                                        h_sb[:, j * 128:(j + 1) * 128],
                                        ident[:64, :64])
                st["hp_ps"] = hp_ps

            def p1_sq(st):
                t = st["t"]
                hp = st["hp_ps"]
                hcf = hc_all[:, t].rearrange("p a b -> p (a b)")
                nc.vector.tensor_copy(hcf, hp.rearrange("p a b -> p (a b)"))
                sqd = wp.tile([128, NCH, 64], BF16, tag="sqd")
                nc.vector.tensor_mul(
                    sqd.rearrange("p a b -> p (a b)"), hcf, hcf)
                nc.vector.reduce_sum(out=s2_all[:, t], in_=sqd, axis=AX.X)

            P1_STAGES = [p1_fc1, p1_evac, p1_fwdT, p1_sq]

            def newton_all():
                s2f = s2_all.rearrange("p a b -> p (a b)")
                vp, yv, u_t = vp_all, rstd_all, u_all
                nc.vector.tensor_scalar(vp, s2f, 1.0 / 64.0, EPS,
                                        op0=OP.mult, op1=OP.add)
                nc.vector.tensor_scalar(yv.bitcast(I32), vp.bitcast(I32), 1,
                                        None, op0=OP.arith_shift_right)
                nc.vector.tensor_scalar(yv.bitcast(I32), yv.bitcast(I32),
                                        0xFFFFFFFF, None, op0=OP.bitwise_xor)
                nc.vector.tensor_scalar(yv.bitcast(I32), yv.bitcast(I32),
                                        MAGIC + 1, None, op0=OP.add)
                for _ in range(3):
                    nc.vector.tensor_mul(u_t, yv, yv)
                    nc.vector.tensor_mul(u_t, u_t, vp)
                    nc.vector.tensor_scalar(u_t, u_t, -0.5, 1.5,
                                            op0=OP.mult, op1=OP.add)
                    nc.vector.tensor_mul(yv, yv, u_t)
                nc.vector.tensor_copy(rstd_bf, rstd_all)

            # ---------------- phase 2 stages ----------------
            def e_apply(st):
                t = st["t"]
                hcn = wp.tile([128, NCH, 64], BF16, tag="hcn")
                rb = rstd_bf[:, t * NCH:(t + 1) * NCH]
                rb_bc = bass.AP(tensor=rb.tensor, offset=rb.offset,
                                ap=[rb.ap[0], rb.ap[-1], [0, 64]])
                nc.gpsimd.tensor_tensor(out=hcn, in0=hc_all[:, t],
                                        in1=rb_bc, op=OP.mult)
                st["hcn"] = hcn
                # prefetch positions into the hg buffer
                img, ti = st["img"], st["ti"]
                p0 = ti * TILE
                hg_t = st["hg"]
                nc.sync.dma_start(out=hg_t[64:76, 0, :],
                                  in_=posq[img, :, p0:p0 + TILE])

            def e_bwdT(st):
                ycm = psb.tile([64, TILE], BF16, tag="tp")
                hcn = st["hcn"]
                for j in range(NCH):
                    nc.tensor.transpose(ycm[:, j * 128:(j + 1) * 128],
                                        hcn[:, j], ident)
                st["ycm"] = ycm

            def e_gelu(st):
                hg_t = st["hg"]
                nc.scalar.activation(hg_t[0:64, 0, :], st["ycm"], AF.Gelu,
                                     bias=bln_t, scale=gcol_t)

            def e_enc0(st):
                e0 = ps.tile([128, TILE], F32, tag="big")
                hg_t = st["hg"]
                for half in range(TILE // 512):
                    sl = slice(half * 512, half * 512 + 512)
                    nc.tensor.matmul(e0[:, sl], we0_t,
                                     dr_rhs(hg_t, 0, 1, sl),
                                     start=True, stop=True,
                                     perf_mode=PM.DoubleRow)
                st["e0"] = e0

            def e_x1(st):
                x1_t = st["x1"]
                nc.scalar.activation(x1_t[:, 0, :], st["e0"], AF.Relu,
                                     scale=s0c_t)

            def e_enc1(st):
                e1 = ps.tile([128, TILE], F32, tag="big")
                x1_t = st["x1"]
                for half in range(TILE // 512):
                    sl = slice(half * 512, half * 512 + 512)
                    nc.tensor.matmul(e1[:, sl], we1_t, dr_rhs(x1_t, 0, 1, sl),
                                     start=True, stop=True,
                                     perf_mode=PM.DoubleRow)
                st["e1"] = e1

            def e_x2(st):
                x2_t = st["x2"]
                nc.vector.tensor_scalar(x2_t[:, 0, :], st["e1"], s1c_t, 0.0,
                                        op0=OP.mult, op1=OP.max)

            def e_enc2(st):
                e2 = ps.tile([128, TILE], F32, tag="big")
                x2_t = st["x2"]
                for half in range(TILE // 512):
                    sl = slice(half * 512, half * 512 + 512)
                    nc.tensor.matmul(e2[:, sl], we2_t, dr_rhs(x2_t, 0, 1, sl),
                                     start=True, stop=True,
                                     perf_mode=PM.DoubleRow)
                st["e2"] = e2

            def e_loc(st):
                img, ti = st["img"], st["ti"]
                loc_t = st["loc"]
                e2 = st["e2"]
                # split the evacuation S/V for balance
                nc.scalar.activation(loc_t[:, ti, 0:512], e2[:, 0:512],
                                     AF.Relu, scale=s2c_t)
                nc.vector.tensor_scalar(loc_t[:, ti, 512:TILE],
                                        e2[:, 512:TILE], s2c_t, 0.0,
                                        op0=OP.mult, op1=OP.max)
                nc.vector.reduce_max(out=gparts[:, img, ti:ti + 1],
                                     in_=e2, axis=AX.X)

            E_STAGES = [e_apply, e_bwdT, e_gelu, e_enc0, e_x1, e_enc1, e_x2,
                        e_enc2, e_loc]

            def glob_s0(sh):
                img = sh["img"]
                graw = wp.tile([128, 1], F32, tag="graw")
                nc.vector.reduce_max(out=graw, in_=gparts[:, img], axis=AX.X)
                gbf = wp.tile([128, 1], BF16, tag="gbf")
                nc.scalar.activation(gbf, graw, AF.Relu, scale=s2c_t)
                gmm = psb.tile([128, NCH], F32, tag="wz", name="gmm")
                nc.tensor.matmul(gmm[:, 0:1], w0b_t, gbf,
                                 start=True, stop=True)
                b0h = wp.tile([128, 1], F32, tag="b0h")
                nc.vector.tensor_scalar(b0h, gmm[:, 0:1], bh0c_t, None,
                                        op0=OP.add)
                sh["b0h"] = b0h

            def h_h0(st):
                h0 = ps.tile([128, TILE], F32, tag="big")
                loc_t = st["loc"]
                ti = st["ti"]
                for half in range(TILE // 512):
                    sl = slice(half * 512, half * 512 + 512)
                    nc.tensor.matmul(h0[:, sl], w0a_t,
                                     dr_rhs(loc_t, ti, NT, sl),
                                     start=True, stop=True,
                                     perf_mode=PM.DoubleRow)
                st["h0"] = h0

            def h_y0(st):
                y0_t = st["y0"]
                nc.scalar.activation(y0_t[:, 0, :], st["h0"], AF.Relu,
                                     bias=st["sh"]["b0h"], scale=s3c_t)

            def h_h1(st):
                # point-major h1: data-stationary DoubleRow, out [128pt, 64f]
                h1 = ps.tile([128, NCH, 64], F32, tag="big", name="h1pm")
                y0_t = st["y0"]
                for j in range(NCH):
                    sl = slice(j * 128, (j + 1) * 128)
                    nc.tensor.matmul(h1[:, j], dr_rhs(y0_t, 0, 1, sl), wh1_t,
                                     start=True, stop=True,
                                     perf_mode=PM.DoubleRow)
                st["h1"] = h1

            def h_y1(st):
                y1_t = st["y1"]
                nc.vector.tensor_scalar(
                    y1_t.rearrange("p a b -> p (a b)"),
                    st["h1"].rearrange("p a b -> p (a b)"), 0.0, None,
                    op0=OP.max)

            def h_dots(st):
                # wz[point] = sum_f w2'[f] * y1_pm[point, f]; raw wz ships
                # to the host (which applies the sigmoid).
                img, ti = st["img"], st["ti"]
                y1_t = st["y1"]
                w2bc = bass.AP(tensor=w2r_t.tensor, offset=w2r_t.offset,
                               ap=[w2r_t.ap[0], [0, NCH], w2r_t.ap[-1]])
                dtt = wp.tile([128, NCH, 64], BF16, tag="dtt")
                nc.gpsimd.tensor_tensor(out=dtt, in0=y1_t, in1=w2bc,
                                        op=OP.mult)
                wz_sb = wp.tile([128, NCH], F32, tag="wzs")
                nc.vector.reduce_sum(out=wz_sb, in_=dtt, axis=AX.X)
                nc.sync.dma_start(out=out[img, ti], in_=wz_sb)

            H_STAGES = [h_h0, h_y0, h_h1, h_y1, h_dots]

            # ---------------- schedule ----------------
            def run_window(units, W=2):
                active = []
                idx = 0
                while idx < len(units) or active:
                    while len(active) < W and idx < len(units):
                        stages, st = units[idx]
                        active.append([stages, st, 0])
                        idx += 1
                    for u in list(active):
                        stages, st, k = u
                        stages[k](st)
                        u[2] += 1
                        if u[2] >= len(stages):
                            active.remove(u)

            # phase 1 with deep DMA prefetch
            p1_sts = [{"img": img, "ti": ti, "t": img * NT + ti}
                      for img in range(BL) for ti in range(NT)]
            for st in p1_sts[:4]:
                p1_load(st)
            for i in range(0, len(p1_sts), 2):
                pair = p1_sts[i:i + 2]
                for st in p1_sts[i + 4:i + 6]:
                    p1_load(st)
                for stg in P1_STAGES:
                    for st in pair:
                        stg(st)

            newton_all()

            # software-pipeline across images: enc(img) runs alongside
            # head(img-1) so the per-image glob barrier never drains the
            # window.
            shs = [{"img": img} for img in range(BL)]
            e_units = {img: [] for img in range(BL)}
            h_units = {img: [] for img in range(BL)}
            for img in range(BL):
                loc_t = loc_b[img % 2]
                for ti in range(NT):
                    k = (img * NT + ti) % 2
                    st = {"img": img, "ti": ti, "t": img * NT + ti,
                          "sh": shs[img], "hg": hg_b[k], "x1": x1_b[k],
                          "x2": x2_b[k], "y0": y0_b[k], "y1": y1_b[k],
                          "loc": loc_t}
                    e_units[img].append((E_STAGES, st))
                    h_units[img].append((H_STAGES, st))
            units = []
            for img in range(BL + 1):
                for ti in range(NT):
                    if img < BL:
                        units.append(e_units[img][ti])
                    if img >= 1:
                        units.append(h_units[img - 1][ti])
                if img < BL:
                    units.append(([glob_s0], shs[img]))
            run_window(units, W=2)

    nc.compile()
    return nc


_CACHE = {}


def _get_nc():
    if "nc" not in _CACHE:
        _CACHE["nc"] = build()
    return _CACHE["nc"]


def _hartley(pts):
    pts = pts.astype(np.float32)
    centroid = pts.mean(axis=1, keepdims=True)
    pc = pts - centroid
    dist = np.sqrt(np.clip((pc ** 2).sum(-1), 0.0, None))
    mean_dist = dist.mean(axis=1, keepdims=True)
    scale = np.float32(np.sqrt(2.0)) / np.clip(mean_dist, 0.001, None)
    scale = np.where(mean_dist < 0.001, np.ones_like(scale), scale)
    pts_norm = pc * scale[..., None]
    return (pts_norm.astype(np.float32), scale[:, 0].astype(np.float32),
            centroid[:, 0, 0].astype(np.float32),
            centroid[:, 0, 1].astype(np.float32))


def _pow2(x):
    """Largest power of two <= x (elementwise, safe)."""
    x = np.maximum(np.asarray(x, np.float64), 1e-30)
    return np.exp2(np.floor(np.log2(x))).astype(np.float32)


def _rowquant(Wrow_mats, target=120.0):
    """Per-row pow2 scale k for a list of [out, in_i] f32 matrices sharing
    rows. Returns (list of fp8 matrices scaled by k, inv_scale [out] f32)."""
    mx = np.zeros(Wrow_mats[0].shape[0], np.float64)
    for M in Wrow_mats:
        if M.size:
            mx = np.maximum(mx, np.abs(M).max(axis=1))
    k = _pow2(target / np.maximum(mx, 1e-30))
    k = np.clip(k, 2.0 ** -30, 2.0 ** 30).astype(np.float32)
    q = [np.clip(M * k[:, None], -240, 240).astype(E4) for M in Wrow_mats]
    return q, (1.0 / k).astype(np.float32)


def _fp8_bias_rows(b, k):
    """bias*k split into fp8 hi+lo rows."""
    z = np.clip(b * k, -240 * 1.99, 240 * 1.99).astype(np.float64)
    hi = np.clip(z, -240, 240).astype(E4)
    lo = np.clip(z - hi.astype(np.float64), -240, 240).astype(E4)
    return hi, lo


def kernel(pos_A, pos_B, feat_A, feat_B,
           fc_w1, fc_b1, fc_ln_g, fc_ln_b, fc_w2, fc_b2,
           enc_w0, enc_g0, enc_b0, enc_w1, enc_g1, enc_b1,
           enc_w2, enc_g2, enc_b2,
           head_w0, head_g0, head_b0, head_w1, head_g1, head_b1,
           head_w2, head_b2):
    f32 = np.float32
    pos_A = np.asarray(pos_A, f32)
    pos_B = np.asarray(pos_B, f32)
    fA = np.asarray(feat_A, f32)
    fB = np.asarray(feat_B, f32)

    # ---- folded f32 weights ----
    bnsc = f32(1.0 / np.sqrt(1.0 + EPS))
    w1c = (fc_w1 - fc_w1.mean(axis=0, keepdims=True)).astype(f32)
    b1cv = (fc_b1 - fc_b1.mean()).astype(f32)
    enc_w0s = (enc_w0 * (enc_g0 * bnsc)[:, None]).astype(f32)
    enc_w1s = (enc_w1 * (enc_g1 * bnsc)[:, None]).astype(f32)
    enc_w2s = (enc_w2 * (enc_g2 * bnsc)[:, None]).astype(f32)
    head_w0s = (head_w0 * (head_g0 * bnsc)[:, None]).astype(f32)
    head_w1s = (head_w1 * (head_g1 * bnsc)[:, None]).astype(f32)
    wfold = (enc_w0s[:, 4:36] @ fc_w2).astype(f32)          # [128, 64]
    benc0 = (enc_b0 + enc_w0s[:, 4:36] @ fc_b2).astype(f32)
    wpos = enc_w0s[:, 0:4].astype(f32)                      # [128, 4]
    w0a_f = head_w0s[:, 0:128]
    w0b_f = head_w0s[:, 128:256]

    # ---- input featurization ----
    d_f = np.abs(fA - fB)                                   # [B, N, C]
    m_f = fA * fB
    ad = float(_pow2(200.0 / max(d_f.max(), 1e-9)))
    am = float(_pow2(200.0 / max(np.abs(m_f).max(), 1e-9)))

    # ---- calibration on two images (f32 forward) ----
    def fwd(img):
        di, mi = d_f[img], m_f[img]
        h = np.concatenate([di, mi], -1) @ w1c.T + b1cv
        var = (h * h).mean(-1, keepdims=True)
        hn = h / np.sqrt(var + EPS) * fc_ln_g + fc_ln_b
        from scipy.special import erf
        hg = hn * 0.5 * (1.0 + erf(hn / np.sqrt(2.0)))
        e0 = hg @ wfold.T + np.concatenate([pos_A[img], pos_B[img]],
                                           -1) @ wpos.T + benc0
        x1 = np.maximum(e0, 0)
        x2 = np.maximum(x1 @ enc_w1s.T + enc_b1, 0)
        loc = np.maximum(x2 @ enc_w2s.T + enc_b2, 0)
        glob = loc.max(axis=0)
        b0h = head_b0 + w0b_f @ glob
        y0 = np.maximum(loc @ w0a_f.T + b0h, 0)
        y1 = np.maximum(y0 @ head_w1s.T + head_b1, 0)
        return (x1.max(0), x2.max(0), loc.max(0), y0.max(0), y1.max(0))

    mxs = [np.maximum(a, b) for a, b in zip(fwd(0), fwd(B // 2))]
    tgt = 48.0
    a1 = _pow2(tgt / np.maximum(mxs[0].max(), 1e-9))
    a2 = _pow2(tgt / np.maximum(mxs[1].max(), 1e-9))
    al = _pow2(tgt / np.maximum(mxs[2].max(), 1e-9))
    a3 = _pow2(tgt / np.maximum(mxs[3].max(), 1e-9))

    # ---- quantized params ----
    (w1d_q, w1m_q), inv1 = _rowquant([w1c[:, 0:128] / ad,
                                      w1c[:, 128:256] / am])
    w1q_h = np.zeros((128, 2, 64), E4)
    w1q_h[:, 0, :] = w1d_q.T
    w1q_h[:, 1, :] = w1m_q.T

    (we0_q,), inv0 = _rowquant(
        [np.concatenate([wfold, wpos * 4, wpos * 4, wpos * 4], axis=1)])
    k0 = 1.0 / inv0
    b0hi, b0lo = _fp8_bias_rows(benc0, k0)
    we0_h = np.zeros((76, 2, 128), E4)
    we0_h[:, 0, :] = we0_q.T
    we0_h[0, 1, :] = b0hi
    we0_h[1, 1, :] = b0lo

    (we1_q,), invs1 = _rowquant([enc_w1s / a1])
    b1hi, b1lo = _fp8_bias_rows(enc_b1, 1.0 / invs1)
    we1_h = np.zeros((128, 2, 128), E4)
    we1_h[:, 0, :] = we1_q.T
    we1_h[0, 1, :] = b1hi
    we1_h[1, 1, :] = b1lo

    (we2_q,), invs2 = _rowquant([enc_w2s / a2])
    b2hi, b2lo = _fp8_bias_rows(enc_b2, 1.0 / invs2)
    we2_h = np.zeros((128, 2, 128), E4)
    we2_h[:, 0, :] = we2_q.T
    we2_h[0, 1, :] = b2hi
    we2_h[1, 1, :] = b2lo

    (w0a_q,), invs3 = _rowquant([w0a_f / al])
    w0a_h = np.zeros((128, 2, 128), E4)
    w0a_h[:, 0, :] = w0a_q.T

    (wh1_q,), invs4 = _rowquant([head_w1s / a3])
    bh1hi, bh1lo = _fp8_bias_rows(head_b1, 1.0 / invs4)
    wh1_h = np.zeros((128, 2, 64), E4)
    wh1_h[:, 0, :] = wh1_q.T
    wh1_h[0, 1, :] = bh1hi
    wh1_h[1, 1, :] = bh1lo

    params = {
        "w1q": w1q_h,
        "sfc": inv1.reshape(64, 1),
        "b1c": b1cv.reshape(64, 1),
        "gcol": fc_ln_g.astype(f32).reshape(64, 1),
        "bln": fc_ln_b.astype(f32).reshape(64, 1),
        "we0": we0_h, "s0c": (inv0 * a1).reshape(128, 1).astype(f32),
        "we1": we1_h, "s1c": (invs1 * a2).reshape(128, 1).astype(f32),
        "we2": we2_h, "s2c": (invs2 * al).reshape(128, 1).astype(f32),
        "w0a": w0a_h, "s3c": (invs3 * a3).reshape(128, 1).astype(f32),
        "w0b": (w0b_f * (a3 / al)).T.astype(BF),
        "bh0c": (head_b0 * a3).reshape(128, 1).astype(f32),
        "wh1": wh1_h,
        "w2r": np.tile((head_w2.reshape(64) * invs4)[None, :],
                       (128, 1)).astype(BF),
    }

    # zero/ones plane: partitions 0/1 = 1.0, rest 0
    zp = np.zeros((128, TILE), f32)
    zp[0:2] = 1.0
    params["zpl"] = zp.astype(E4)

    # ---- per-core data tensors ----
    # d/m packed [B, 128, 2, N] fp8
    dmh = np.empty((B, C, 2, N), E4)
    dmh[:, :, 0, :] = np.clip(d_f * ad, 0, 240).transpose(0, 2, 1).astype(E4)
    dmh[:, :, 1, :] = np.clip(m_f * am, -240, 240).transpose(0, 2, 1).astype(E4)
    # but the weights were built for d_true = d_stored/ad with stored=d*ad:
    # inv1 built from w/ad so PSUM = k*(w.d_true*ad/ad)... handled above.

    # positions hi/mid/lo fp8, scaled by 1/4
    psc = np.concatenate([pos_A, pos_B], -1).transpose(0, 2, 1) * 0.25
    hi = np.clip(psc, -240, 240).astype(E4)
    r1 = psc - hi.astype(f32)
    mid = np.clip(r1, -240, 240).astype(E4)
    r2 = r1 - mid.astype(f32)
    lo = np.clip(r2, -240, 240).astype(E4)
    posq_h = np.concatenate([hi, mid, lo], axis=1)          # [B, 12, N]

    # q monomials (Hartley-normalized)
    srcn, sA, cxA, cyA = _hartley(pos_A)
    dstn, sB, cxB, cyB = _hartley(pos_B)
    sx, sy = srcn[..., 0], srcn[..., 1]
    dx, dy = dstn[..., 0], dstn[..., 1]
    one = np.ones_like(sx)
    q9 = np.stack([sx, sy, one, dx, dy, dx * sx, dx * sy, dy * sx, dy * sy],
                  axis=-1)                                   # [B, N, 9]

    in_maps = []
    for i in range(NCORES):
        sl = slice(i * BL, (i + 1) * BL)
        mcore = {"dm": np.ascontiguousarray(dmh[sl]),
                 "posq": np.ascontiguousarray(posq_h[sl])}
        mcore.update(params)
        in_maps.append(mcore)

    nc = _get_nc()
    res = bass_utils.run_bass_kernel_spmd(nc, in_maps,
                                          core_ids=list(range(NCORES)))
    wzr = np.concatenate([res.results[i]["out"] for i in range(NCORES)],
                         axis=0)                  # [B, NT, 128, NCH]
    wz = wzr.transpose(0, 1, 3, 2).reshape(B, N).astype(np.float64)
    w = 1.0 / (1.0 + np.exp(-(wz + float(head_b2[0]))))
    w = w.astype(f32)
    qw = q9 * w[:, :, None]
    M = np.einsum('bnk,bnl->bkl', qw, q9, optimize=True).astype(f32)

    # ---- host post: assemble AtWA/AtWb, solve, compose ----
    u3 = [0, 1, 2]
    AtWA = np.zeros((B, 8, 8), f32)
    AtWA[:, 0:3, 0:3] = M[:, 0:3, 0:3]
    AtWA[:, 3:6, 3:6] = M[:, 0:3, 0:3]
    AtWA[:, 0:3, 6] = -M[:, u3, 5]
    AtWA[:, 0:3, 7] = -M[:, u3, 6]
    AtWA[:, 3:6, 6] = -M[:, u3, 7]
    AtWA[:, 3:6, 7] = -M[:, u3, 8]
    AtWA[:, 6, 0:3] = -M[:, u3, 5]
    AtWA[:, 7, 0:3] = -M[:, u3, 6]
    AtWA[:, 6, 3:6] = -M[:, u3, 7]
    AtWA[:, 7, 3:6] = -M[:, u3, 8]
    AtWA[:, 6, 6] = M[:, 5, 5] + M[:, 7, 7]
    AtWA[:, 6, 7] = M[:, 5, 6] + M[:, 7, 8]
    AtWA[:, 7, 6] = M[:, 6, 5] + M[:, 8, 7]
    AtWA[:, 7, 7] = M[:, 6, 6] + M[:, 8, 8]
    AtWb = np.zeros((B, 8), f32)
    AtWb[:, 0:3] = M[:, 3, 0:3]
    AtWb[:, 3:6] = M[:, 4, 0:3]
    AtWb[:, 6] = -(M[:, 3, 5] + M[:, 4, 7])
    AtWb[:, 7] = -(M[:, 3, 6] + M[:, 4, 8])
    AtWA += REG * np.eye(8, dtype=f32)[None]
    h_id = np.array([1, 0, 0, 0, 1, 0, 0, 0], f32)
    AtWb += REG * h_id[None]

    try:
        h8 = np.linalg.solve(AtWA, AtWb[..., None])[..., 0].astype(f32)
    except np.linalg.LinAlgError:
        h8 = np.zeros((B, 8), f32)
        for b in range(B):
            try:
                h8[b] = np.linalg.solve(AtWA[b], AtWb[b])
            except np.linalg.LinAlgError:
                h8[b] = np.nan
    finite = np.all(np.isfinite(h8), axis=-1, keepdims=True)
    h8 = np.where(finite, h8, h_id[None])
    H_norm = np.concatenate([h8, np.ones((B, 1), f32)], axis=-1)
    H_norm = H_norm.reshape(B, 3, 3)

    T_src = np.zeros((B, 3, 3), f32)
    T_src[:, 0, 0] = sA
    T_src[:, 1, 1] = sA
    T_src[:, 0, 2] = -sA * cxA
    T_src[:, 1, 2] = -sA * cyA
    T_src[:, 2, 2] = 1.0
    s_dst = np.clip(sB, 1e-6, None)
    T_dst_inv = np.zeros((B, 3, 3), f32)
    T_dst_inv[:, 0, 0] = 1.0 / s_dst
    T_dst_inv[:, 1, 1] = 1.0 / s_dst
    T_dst_inv[:, 0, 2] = (sB * cxB) / s_dst
    T_dst_inv[:, 1, 2] = (sB * cyB) / s_dst
    T_dst_inv[:, 2, 2] = 1.0

    H = (T_dst_inv @ (H_norm @ T_src)).astype(f32)
    H = H / np.clip(np.abs(H[:, 2:3, 2:3]), 1e-8, None)
    h33 = H[:, 2:3, 2:3]
    sgn = np.sign(h33)
    sgn = np.where(sgn == 0, np.ones_like(sgn), sgn)
    H = H / (np.clip(np.abs(h33), 1e-8, None) * sgn)
    H_finite = np.all(np.isfinite(H), axis=(-2, -1))
    a33 = np.abs(H[:, 2, 2])
    valid = H_finite & (a33 > 1e-4) & (a33 < 1e4)
    eye = np.eye(3, dtype=f32)
    H = np.where(valid[:, None, None], H, eye[None])
    return H.astype(f32)


# revision 37
# speedup vs baseline: 1.3756x; 1.0286x over previous
"""AgriMatcher Trainium2 kernel: point-matching network + weighted-DLT homography.

Data-parallel over batch B=64 across 8 NeuronCores (8 images/core). The device
runs the network (fc-compression + LayerNorm + gelu, PointNet encoder, weight
head) and accumulates the per-image 9x9 weighted Gram matrix
M = sum_n w_n q_n q_n^T over Hartley-normalized point monomials q (host-built).
Host assembles AtWA/AtWb from M, solves 8x8, composes the 3x3 homographies.

Perf structure:
- fc1 and all five 128-wide layers run as fp8(E4M3) DoubleRow matmuls
  (2 fp8 rows/PE-cell = 2x-4x tensor throughput). Weights carry per-row pow2
  scales, undone by each evacuation's per-partition scale; layer biases ride a
  static ones-row in the DoubleRow zero-plane.
- Host precomputes |fA-fB| and fA*fB (fp8, pow2-scaled), the DLT q monomials,
  and a 3-way fp8 hi/mid/lo split of the positions for the encoder input.
- LayerNorm via PE transposes; variance by fused square+accumulate
  (scalar_tensor_tensor) on Vector; rstd (fast-invsqrt + Newton) on Vector;
  per-chunk rstd apply on GpSimd.
- Evacuations (PSUM->SBUF w/ relu+scale) balanced across Scalar and Vector.
"""

import numpy as np
import ml_dtypes

import concourse.bass as bass
import concourse.mybir as mybir
import concourse.tile as tile
from concourse import bacc, bass_utils
from concourse.masks import make_identity

F32 = mybir.dt.float32
BF16 = mybir.dt.bfloat16
FP8 = mybir.dt.float8e4
I32 = mybir.dt.int32
AF = mybir.ActivationFunctionType
OP = mybir.AluOpType
AX = mybir.AxisListType
PM = mybir.MatmulPerfMode

B, N, C = 64, 4096, 128
HID, COMP = 128, 32
NCORES = 8
BL = B // NCORES          # images per core
TILE = 1024               # points per tile
NT = N // TILE            # tiles per image (4)
NCH = TILE // 128         # 128-pt chunks per tile (8)
NTC = BL * NT             # tiles per core (32)
NC32 = N // 128           # 128-pt chunks per image (32)
EPS = 1e-5
REG = 1e-4
MAGIC = 0x5F3759DF

BF = ml_dtypes.bfloat16
E4 = ml_dtypes.float8_e4m3


def build():
    nc = bacc.Bacc("TRN2", target_bir_lowering=False, debug=False,
                   num_devices=NCORES)

    dm = nc.dram_tensor("dm", [BL, 128, 2, N], FP8, kind="ExternalInput").ap()
    posq = nc.dram_tensor("posq", [BL, 12, N], FP8, kind="ExternalInput").ap()
    zpl = nc.dram_tensor("zpl", [128, TILE], FP8, kind="ExternalInput").ap()
    # params
    w1q = nc.dram_tensor("w1q", [128, 2, 64], FP8, kind="ExternalInput").ap()
    sfc = nc.dram_tensor("sfc", [64, 1], F32, kind="ExternalInput").ap()
    b1c = nc.dram_tensor("b1c", [64, 1], F32, kind="ExternalInput").ap()
    gcol = nc.dram_tensor("gcol", [64, 1], F32, kind="ExternalInput").ap()
    bln = nc.dram_tensor("bln", [64, 1], F32, kind="ExternalInput").ap()
    we0 = nc.dram_tensor("we0", [76, 2, 128], FP8, kind="ExternalInput").ap()
    s0c = nc.dram_tensor("s0c", [128, 1], F32, kind="ExternalInput").ap()
    we1 = nc.dram_tensor("we1", [128, 2, 128], FP8, kind="ExternalInput").ap()
    s1c = nc.dram_tensor("s1c", [128, 1], F32, kind="ExternalInput").ap()
    we2 = nc.dram_tensor("we2", [128, 2, 128], FP8, kind="ExternalInput").ap()
    s2c = nc.dram_tensor("s2c", [128, 1], F32, kind="ExternalInput").ap()
    w0a = nc.dram_tensor("w0a", [128, 2, 128], FP8, kind="ExternalInput").ap()
    s3c = nc.dram_tensor("s3c", [128, 1], F32, kind="ExternalInput").ap()
    w0b = nc.dram_tensor("w0b", [128, 128], BF16, kind="ExternalInput").ap()
    bh0c = nc.dram_tensor("bh0c", [128, 1], F32, kind="ExternalInput").ap()
    wh1 = nc.dram_tensor("wh1", [128, 2, 64], FP8, kind="ExternalInput").ap()
    w2r = nc.dram_tensor("w2r", [128, 64], BF16, kind="ExternalInput").ap()

    out = nc.dram_tensor("out", [BL, NT, 128, NCH], F32,
                         kind="ExternalOutput").ap()

    with tile.TileContext(nc) as tc:
        with (
            tc.tile_pool(name="const", bufs=1) as cp,
            tc.tile_pool(name="persist", bufs=1) as pp,
            tc.tile_pool(name="work", bufs=3) as wp,
            tc.tile_pool(name="feat", bufs=4) as fp,
            tc.tile_pool(name="ps", bufs=2, space="PSUM") as ps,
            tc.tile_pool(name="psb", bufs=2, space="PSUM") as psb,
        ):
            ident = cp.tile([128, 128], BF16)
            make_identity(nc, ident)

            def cload(ap_in, shape, dtype):
                t = cp.tile(shape, dtype, tag=ap_in.tensor.name)
                nc.sync.dma_start(out=t, in_=ap_in)
                return t

            w1q_t = cload(w1q, [128, 2, 64], FP8)
            sfc_t = cload(sfc, [64, 1], F32)
            b1c_t = cload(b1c, [64, 1], F32)
            gcol_t = cload(gcol, [64, 1], F32)
            bln_t = cload(bln, [64, 1], F32)
            we0_t = cload(we0, [76, 2, 128], FP8)
            s0c_t = cload(s0c, [128, 1], F32)
            we1_t = cload(we1, [128, 2, 128], FP8)
            s1c_t = cload(s1c, [128, 1], F32)
            we2_t = cload(we2, [128, 2, 128], FP8)
            s2c_t = cload(s2c, [128, 1], F32)
            w0a_t = cload(w0a, [128, 2, 128], FP8)
            s3c_t = cload(s3c, [128, 1], F32)
            w0b_t = cload(w0b, [128, 128], BF16)
            bh0c_t = cload(bh0c, [128, 1], F32)
            wh1_t = cload(wh1, [128, 2, 64], FP8)
            w2r_t = cload(w2r, [128, 64], BF16)

            # persistent state
            hc_all = pp.tile([128, NTC, NCH, 64], BF16)
            s2_all = pp.tile([128, NTC, NCH], F32)
            rstd_all = pp.tile([128, NTC * NCH], F32)
            rstd_bf = pp.tile([128, NTC * NCH], BF16)
            vp_all = pp.tile([128, NTC * NCH], F32)
            u_all = pp.tile([128, NTC * NCH], F32)
            gparts = pp.tile([128, BL, NT], F32)

            # fp8 activation tiles: [*, 2, TILE], plane 1 = zero pad with
            # ones at partitions 0/1 (DoubleRow bias rows). DMA'd once.
            hg_b = [pp.tile([76, 2, TILE], FP8, tag=f"hg{i}", name=f"hg{i}")
                    for i in range(2)]
            x1_b = [pp.tile([128, 2, TILE], FP8, tag=f"x1{i}", name=f"x1{i}")
                    for i in range(2)]
            x2_b = [pp.tile([128, 2, TILE], FP8, tag=f"x2{i}", name=f"x2{i}")
                    for i in range(2)]
            y0_b = [pp.tile([128, 2, TILE], FP8, tag=f"y0{i}", name=f"y0{i}")
                    for i in range(2)]
            # local: plane 0..3 = data tiles, plane 4 = zero pad
            loc_b = [pp.tile([128, NT + 1, TILE], FP8, tag=f"lc{i}",
                             name=f"lc{i}") for i in range(2)]
            y1_b = [pp.tile([128, NCH, 64], BF16, tag=f"y1{i}", name=f"y1{i}")
                    for i in range(2)]

            for t in hg_b:
                nc.sync.dma_start(out=t[0:76, 1, :], in_=zpl[0:76, :])
            for t in x1_b + x2_b + y0_b:
                nc.sync.dma_start(out=t[:, 1, :], in_=zpl)
            for t in loc_b:
                nc.sync.dma_start(out=t[:, NT, :], in_=zpl)

            def dr_rhs(t, plane, zplane, sl):
                base = t[:, plane, sl]
                return bass.AP(tensor=base.tensor, offset=base.offset,
                               ap=[base.ap[0],
                                   [(zplane - plane) * TILE, 2],
                                   base.ap[-1]])

            # ---------------- phase 1: fc1 + LN stats ----------------
            def p1_load(st):
                img, ti = st["img"], st["ti"]
                p0 = ti * TILE
                dm_t = fp.tile([128, 2, TILE], FP8, tag="dm")
                nc.sync.dma_start(out=dm_t, in_=dm[img, :, :, p0:p0 + TILE])
                st["dm"] = dm_t

            def p1_fc1(st):
                h_ps = ps.tile([64, TILE], F32, tag="big")
                dm_t = st["dm"]
                for half in range(TILE // 512):
                    sl = slice(half * 512, half * 512 + 512)
                    nc.tensor.matmul(h_ps[:, sl], w1q_t, dm_t[:, :, sl],
                                     start=True, stop=True,
                                     perf_mode=PM.DoubleRow)
                st["h_ps"] = h_ps

            def p1_evac(st):
                h_sb = wp.tile([64, TILE], BF16, tag="h_sb")
                nc.scalar.activation(h_sb, st["h_ps"], AF.Identity,
                                     bias=b1c_t, scale=sfc_t)
                st["h_sb"] = h_sb

            def p1_fwdT(st):
                hp_ps = psb.tile([128, NCH, 64], BF16, tag="tp")
                h_sb = st["h_sb"]
                for j in range(NCH):
                    nc.tensor.transpose(hp_ps[:, j, :],
                                        h_sb[:, j * 128:(j + 1) * 128],
                                        ident[:64, :64])
                st["hp_ps"] = hp_ps

            def p1_sq(st):
                t = st["t"]
                hp = st["hp_ps"]
                hcf = hc_all[:, t].rearrange("p a b -> p (a b)")
                nc.vector.tensor_copy(hcf, hp.rearrange("p a b -> p (a b)"))
                sqd = wp.tile([128, NCH, 64], BF16, tag="sqd")
                nc.vector.tensor_mul(
                    sqd.rearrange("p a b -> p (a b)"), hcf, hcf)
                nc.vector.reduce_sum(out=s2_all[:, t], in_=sqd, axis=AX.X)

            P1_STAGES = [p1_fc1, p1_evac, p1_fwdT, p1_sq]

            def newton_all():
                s2f = s2_all.rearrange("p a b -> p (a b)")
                vp, yv, u_t = vp_all, rstd_all, u_all
                nc.vector.tensor_scalar(vp, s2f, 1.0 / 64.0, EPS,
                                        op0=OP.mult, op1=OP.add)
                nc.vector.tensor_scalar(yv.bitcast(I32), vp.bitcast(I32), 1,
                                        None, op0=OP.arith_shift_right)
                nc.vector.tensor_scalar(yv.bitcast(I32), yv.bitcast(I32),
                                        0xFFFFFFFF, None, op0=OP.bitwise_xor)
                nc.vector.tensor_scalar(yv.bitcast(I32), yv.bitcast(I32),
                                        MAGIC + 1, None, op0=OP.add)
                for _ in range(3):
                    nc.vector.tensor_mul(u_t, yv, yv)
                    nc.vector.tensor_mul(u_t, u_t, vp)
                    nc.vector.tensor_scalar(u_t, u_t, -0.5, 1.5,
                                            op0=OP.mult, op1=OP.add)
                    nc.vector.tensor_mul(yv, yv, u_t)
                nc.vector.tensor_copy(rstd_bf, rstd_all)

            # ---------------- phase 2 stages ----------------
            def e_apply(st):
                t = st["t"]
                hcn = wp.tile([128, NCH, 64], BF16, tag="hcn")
                rb = rstd_bf[:, t * NCH:(t + 1) * NCH]
                rb_bc = bass.AP(tensor=rb.tensor, offset=rb.offset,
                                ap=[rb.ap[0], rb.ap[-1], [0, 64]])
                nc.gpsimd.tensor_tensor(out=hcn, in0=hc_all[:, t],
                                        in1=rb_bc, op=OP.mult)
                st["hcn"] = hcn
                # prefetch positions into the hg buffer
                img, ti = st["img"], st["ti"]
                p0 = ti * TILE
                hg_t = st["hg"]
                nc.sync.dma_start(out=hg_t[64:76, 0, :],
                                  in_=posq[img, :, p0:p0 + TILE])

            def e_bwdT(st):
                ycm = psb.tile([64, TILE], BF16, tag="tp")
                hcn = st["hcn"]
                for j in range(NCH):
                    nc.tensor.transpose(ycm[:, j * 128:(j + 1) * 128],
                                        hcn[:, j], ident)
                st["ycm"] = ycm

            def e_gelu(st):
                hg_t = st["hg"]
                nc.scalar.activation(hg_t[0:64, 0, :], st["ycm"], AF.Gelu,
                                     bias=bln_t, scale=gcol_t)

            def e_enc0(st):
                e0 = ps.tile([128, TILE], F32, tag="big")
                hg_t = st["hg"]
                for half in range(TILE // 512):
                    sl = slice(half * 512, half * 512 + 512)
                    nc.tensor.matmul(e0[:, sl], we0_t,
                                     dr_rhs(hg_t, 0, 1, sl),
                                     start=True, stop=True,
                                     perf_mode=PM.DoubleRow)
                st["e0"] = e0

            def e_x1(st):
                x1_t = st["x1"]
                nc.scalar.activation(x1_t[:, 0, :], st["e0"], AF.Relu,
                                     scale=s0c_t)

            def e_enc1(st):
                e1 = ps.tile([128, TILE], F32, tag="big")
                x1_t = st["x1"]
                for half in range(TILE // 512):
                    sl = slice(half * 512, half * 512 + 512)
                    nc.tensor.matmul(e1[:, sl], we1_t, dr_rhs(x1_t, 0, 1, sl),
                                     start=True, stop=True,
                                     perf_mode=PM.DoubleRow)
                st["e1"] = e1

            def e_x2(st):
                x2_t = st["x2"]
                nc.vector.tensor_scalar(x2_t[:, 0, :], st["e1"], s1c_t, 0.0,
                                        op0=OP.mult, op1=OP.max)

            def e_enc2(st):
                e2 = ps.tile([128, TILE], F32, tag="big")
                x2_t = st["x2"]
                for half in range(TILE // 512):
                    sl = slice(half * 512, half * 512 + 512)
                    nc.tensor.matmul(e2[:, sl], we2_t, dr_rhs(x2_t, 0, 1, sl),
                                     start=True, stop=True,
                                     perf_mode=PM.DoubleRow)
                st["e2"] = e2

            def e_loc(st):
                img, ti = st["img"], st["ti"]
                loc_t = st["loc"]
                e2 = st["e2"]
                # split the evacuation S/V for balance
                nc.scalar.activation(loc_t[:, ti, 0:512], e2[:, 0:512],
                                     AF.Relu, scale=s2c_t)
                nc.vector.tensor_scalar(loc_t[:, ti, 512:TILE],
                                        e2[:, 512:TILE], s2c_t, 0.0,
                                        op0=OP.mult, op1=OP.max)
                nc.vector.reduce_max(out=gparts[:, img, ti:ti + 1],
                                     in_=e2, axis=AX.X)

            E_STAGES = [e_apply, e_bwdT, e_gelu, e_enc0, e_x1, e_enc1, e_x2,
                        e_enc2, e_loc]

            def glob_s0(sh):
                img = sh["img"]
                graw = wp.tile([128, 1], F32, tag="graw")
                nc.vector.reduce_max(out=graw, in_=gparts[:, img], axis=AX.X)
                gbf = wp.tile([128, 1], BF16, tag="gbf")
                nc.scalar.activation(gbf, graw, AF.Relu, scale=s2c_t)
                gmm = psb.tile([128, NCH], F32, tag="wz", name="gmm")
                nc.tensor.matmul(gmm[:, 0:1], w0b_t, gbf,
                                 start=True, stop=True)
                b0h = wp.tile([128, 1], F32, tag="b0h")
                nc.vector.tensor_scalar(b0h, gmm[:, 0:1], bh0c_t, None,
                                        op0=OP.add)
                sh["b0h"] = b0h

            def h_h0(st):
                h0 = ps.tile([128, TILE], F32, tag="big")
                loc_t = st["loc"]
                ti = st["ti"]
                for half in range(TILE // 512):
                    sl = slice(half * 512, half * 512 + 512)
                    nc.tensor.matmul(h0[:, sl], w0a_t,
                                     dr_rhs(loc_t, ti, NT, sl),
                                     start=True, stop=True,
                                     perf_mode=PM.DoubleRow)
                st["h0"] = h0

            def h_y0(st):
                y0_t = st["y0"]
                nc.scalar.activation(y0_t[:, 0, :], st["h0"], AF.Relu,
                                     bias=st["sh"]["b0h"], scale=s3c_t)

            def h_h1(st):
                # point-major h1: data-stationary DoubleRow, out [128pt, 64f]
                h1 = ps.tile([128, NCH, 64], F32, tag="big", name="h1pm")
                y0_t = st["y0"]
                for j in range(NCH):
                    sl = slice(j * 128, (j + 1) * 128)
                    nc.tensor.matmul(h1[:, j], dr_rhs(y0_t, 0, 1, sl), wh1_t,
                                     start=True, stop=True,
                                     perf_mode=PM.DoubleRow)
                st["h1"] = h1

            def h_y1(st):
                y1_t = st["y1"]
                nc.vector.tensor_scalar(
                    y1_t.rearrange("p a b -> p (a b)"),
                    st["h1"].rearrange("p a b -> p (a b)"), 0.0, None,
                    op0=OP.max)

            def h_dots(st):
                # wz[point] = sum_f w2'[f] * y1_pm[point, f]; raw wz ships
                # to the host (which applies the sigmoid).
                img, ti = st["img"], st["ti"]
                y1_t = st["y1"]
                w2bc = bass.AP(tensor=w2r_t.tensor, offset=w2r_t.offset,
                               ap=[w2r_t.ap[0], [0, NCH], w2r_t.ap[-1]])
                dtt = wp.tile([128, NCH, 64], BF16, tag="dtt")
                nc.gpsimd.tensor_tensor(out=dtt, in0=y1_t, in1=w2bc,
                                        op=OP.mult)
                wz_sb = wp.tile([128, NCH], F32, tag="wzs")
                nc.vector.reduce_sum(out=wz_sb, in_=dtt, axis=AX.X)
                nc.sync.dma_start(out=out[img, ti], in_=wz_sb)

            H_STAGES = [h_h0, h_y0, h_h1, h_y1, h_dots]

            # ---------------- schedule ----------------
            def run_window(units, W=2):
                active = []
                idx = 0
                while idx < len(units) or active:
                    while len(active) < W and idx < len(units):
                        stages, st = units[idx]
                        active.append([stages, st, 0])
                        idx += 1
                    for u in list(active):
                        stages, st, k = u
                        stages[k](st)
                        u[2] += 1
                        if u[2] >= len(stages):
                            active.remove(u)

            # phase 1 with deep DMA prefetch
            p1_sts = [{"img": img, "ti": ti, "t": img * NT + ti}
                      for img in range(BL) for ti in range(NT)]
            for st in p1_sts[:4]:
                p1_load(st)
            for i in range(0, len(p1_sts), 2):
                pair = p1_sts[i:i + 2]
                for st in p1_sts[i + 4:i + 6]:
                    p1_load(st)
                for stg in P1_STAGES:
                    for st in pair:
                        stg(st)

            newton_all()

            # software-pipeline across images: enc(img) runs alongside
            # head(img-1) so the per-image glob barrier never drains the
            # window.
            shs = [{"img": img} for img in range(BL)]
            e_units = {img: [] for img in range(BL)}
            h_units = {img: [] for img in range(BL)}
            for img in range(BL):
                loc_t = loc_b[img % 2]
                for ti in range(NT):
                    k = (img * NT + ti) % 2
                    st = {"img": img, "ti": ti, "t": img * NT + ti,
                          "sh": shs[img], "hg": hg_b[k], "x1": x1_b[k],
                          "x2": x2_b[k], "y0": y0_b[k], "y1": y1_b[k],
                          "loc": loc_t}
                    e_units[img].append((E_STAGES, st))
                    h_units[img].append((H_STAGES, st))
            units = []
            for img in range(BL + 1):
                for ti in range(NT):
                    if img < BL:
                        units.append(e_units[img][ti])
                    if img >= 1:
                        units.append(h_units[img - 1][ti])
                if img < BL:
                    units.append(([glob_s0], shs[img]))
            run_window(units, W=3)

    nc.compile()
    return nc


_CACHE = {}


def _get_nc():
    if "nc" not in _CACHE:
        _CACHE["nc"] = build()
    return _CACHE["nc"]


def _hartley(pts):
    pts = pts.astype(np.float32)
    centroid = pts.mean(axis=1, keepdims=True)
    pc = pts - centroid
    dist = np.sqrt(np.clip((pc ** 2).sum(-1), 0.0, None))
    mean_dist = dist.mean(axis=1, keepdims=True)
    scale = np.float32(np.sqrt(2.0)) / np.clip(mean_dist, 0.001, None)
    scale = np.where(mean_dist < 0.001, np.ones_like(scale), scale)
    pts_norm = pc * scale[..., None]
    return (pts_norm.astype(np.float32), scale[:, 0].astype(np.float32),
            centroid[:, 0, 0].astype(np.float32),
            centroid[:, 0, 1].astype(np.float32))


def _pow2(x):
    """Largest power of two <= x (elementwise, safe)."""
    x = np.maximum(np.asarray(x, np.float64), 1e-30)
    return np.exp2(np.floor(np.log2(x))).astype(np.float32)


def _rowquant(Wrow_mats, target=120.0):
    """Per-row pow2 scale k for a list of [out, in_i] f32 matrices sharing
    rows. Returns (list of fp8 matrices scaled by k, inv_scale [out] f32)."""
    mx = np.zeros(Wrow_mats[0].shape[0], np.float64)
    for M in Wrow_mats:
        if M.size:
            mx = np.maximum(mx, np.abs(M).max(axis=1))
    k = _pow2(target / np.maximum(mx, 1e-30))
    k = np.clip(k, 2.0 ** -30, 2.0 ** 30).astype(np.float32)
    q = [np.clip(M * k[:, None], -240, 240).astype(E4) for M in Wrow_mats]
    return q, (1.0 / k).astype(np.float32)


def _fp8_bias_rows(b, k):
    """bias*k split into fp8 hi+lo rows."""
    z = np.clip(b * k, -240 * 1.99, 240 * 1.99).astype(np.float64)
    hi = np.clip(z, -240, 240).astype(E4)
    lo = np.clip(z - hi.astype(np.float64), -240, 240).astype(E4)
    return hi, lo


def kernel(pos_A, pos_B, feat_A, feat_B,
           fc_w1, fc_b1, fc_ln_g, fc_ln_b, fc_w2, fc_b2,
           enc_w0, enc_g0, enc_b0, enc_w1, enc_g1, enc_b1,
           enc_w2, enc_g2, enc_b2,
           head_w0, head_g0, head_b0, head_w1, head_g1, head_b1,
           head_w2, head_b2):
    f32 = np.float32
    pos_A = np.asarray(pos_A, f32)
    pos_B = np.asarray(pos_B, f32)
    fA = np.asarray(feat_A, f32)
    fB = np.asarray(feat_B, f32)

    # ---- folded f32 weights ----
    bnsc = f32(1.0 / np.sqrt(1.0 + EPS))
    w1c = (fc_w1 - fc_w1.mean(axis=0, keepdims=True)).astype(f32)
    b1cv = (fc_b1 - fc_b1.mean()).astype(f32)
    enc_w0s = (enc_w0 * (enc_g0 * bnsc)[:, None]).astype(f32)
    enc_w1s = (enc_w1 * (enc_g1 * bnsc)[:, None]).astype(f32)
    enc_w2s = (enc_w2 * (enc_g2 * bnsc)[:, None]).astype(f32)
    head_w0s = (head_w0 * (head_g0 * bnsc)[:, None]).astype(f32)
    head_w1s = (head_w1 * (head_g1 * bnsc)[:, None]).astype(f32)
    wfold = (enc_w0s[:, 4:36] @ fc_w2).astype(f32)          # [128, 64]
    benc0 = (enc_b0 + enc_w0s[:, 4:36] @ fc_b2).astype(f32)
    wpos = enc_w0s[:, 0:4].astype(f32)                      # [128, 4]
    w0a_f = head_w0s[:, 0:128]
    w0b_f = head_w0s[:, 128:256]

    # ---- input featurization ----
    d_f = np.abs(fA - fB)                                   # [B, N, C]
    m_f = fA * fB
    ad = float(_pow2(200.0 / max(d_f.max(), 1e-9)))
    am = float(_pow2(200.0 / max(np.abs(m_f).max(), 1e-9)))

    # ---- calibration on two images (f32 forward) ----
    def fwd(img):
        di, mi = d_f[img], m_f[img]
        h = np.concatenate([di, mi], -1) @ w1c.T + b1cv
        var = (h * h).mean(-1, keepdims=True)
        hn = h / np.sqrt(var + EPS) * fc_ln_g + fc_ln_b
        from scipy.special import erf
        hg = hn * 0.5 * (1.0 + erf(hn / np.sqrt(2.0)))
        e0 = hg @ wfold.T + np.concatenate([pos_A[img], pos_B[img]],
                                           -1) @ wpos.T + benc0
        x1 = np.maximum(e0, 0)
        x2 = np.maximum(x1 @ enc_w1s.T + enc_b1, 0)
        loc = np.maximum(x2 @ enc_w2s.T + enc_b2, 0)
        glob = loc.max(axis=0)
        b0h = head_b0 + w0b_f @ glob
        y0 = np.maximum(loc @ w0a_f.T + b0h, 0)
        y1 = np.maximum(y0 @ head_w1s.T + head_b1, 0)
        return (x1.max(0), x2.max(0), loc.max(0), y0.max(0), y1.max(0))

    mxs = [np.maximum(a, b) for a, b in zip(fwd(0), fwd(B // 2))]
    tgt = 48.0
    a1 = _pow2(tgt / np.maximum(mxs[0].max(), 1e-9))
    a2 = _pow2(tgt / np.maximum(mxs[1].max(), 1e-9))
    al = _pow2(tgt / np.maximum(mxs[2].max(), 1e-9))
    a3 = _pow2(tgt / np.maximum(mxs[3].max(), 1e-9))

    # ---- quantized params ----
    (w1d_q, w1m_q), inv1 = _rowquant([w1c[:, 0:128] / ad,
                                      w1c[:, 128:256] / am])
    w1q_h = np.zeros((128, 2, 64), E4)
    w1q_h[:, 0, :] = w1d_q.T
    w1q_h[:, 1, :] = w1m_q.T

    (we0_q,), inv0 = _rowquant(
        [np.concatenate([wfold, wpos * 4, wpos * 4, wpos * 4], axis=1)])
    k0 = 1.0 / inv0
    b0hi, b0lo = _fp8_bias_rows(benc0, k0)
    we0_h = np.zeros((76, 2, 128), E4)
    we0_h[:, 0, :] = we0_q.T
    we0_h[0, 1, :] = b0hi
    we0_h[1, 1, :] = b0lo

    (we1_q,), invs1 = _rowquant([enc_w1s / a1])
    b1hi, b1lo = _fp8_bias_rows(enc_b1, 1.0 / invs1)
    we1_h = np.zeros((128, 2, 128), E4)
    we1_h[:, 0, :] = we1_q.T
    we1_h[0, 1, :] = b1hi
    we1_h[1, 1, :] = b1lo

    (we2_q,), invs2 = _rowquant([enc_w2s / a2])
    b2hi, b2lo = _fp8_bias_rows(enc_b2, 1.0 / invs2)
    we2_h = np.zeros((128, 2, 128), E4)
    we2_h[:, 0, :] = we2_q.T
    we2_h[0, 1, :] = b2hi
    we2_h[1, 1, :] = b2lo

    (w0a_q,), invs3 = _rowquant([w0a_f / al])
    w0a_h = np.zeros((128, 2, 128), E4)
    w0a_h[:, 0, :] = w0a_q.T

    (wh1_q,), invs4 = _rowquant([head_w1s / a3])
    bh1hi, bh1lo = _fp8_bias_rows(head_b1, 1.0 / invs4)
    wh1_h = np.zeros((128, 2, 64), E4)
    wh1_h[:, 0, :] = wh1_q.T
    wh1_h[0, 1, :] = bh1hi
    wh1_h[1, 1, :] = bh1lo

    params = {
        "w1q": w1q_h,
        "sfc": inv1.reshape(64, 1),
        "b1c": b1cv.reshape(64, 1),
        "gcol": fc_ln_g.astype(f32).reshape(64, 1),
        "bln": fc_ln_b.astype(f32).reshape(64, 1),
        "we0": we0_h, "s0c": (inv0 * a1).reshape(128, 1).astype(f32),
        "we1": we1_h, "s1c": (invs1 * a2).reshape(128, 1).astype(f32),
        "we2": we2_h, "s2c": (invs2 * al).reshape(128, 1).astype(f32),
        "w0a": w0a_h, "s3c": (invs3 * a3).reshape(128, 1).astype(f32),
        "w0b": (w0b_f * (a3 / al)).T.astype(BF),
        "bh0c": (head_b0 * a3).reshape(128, 1).astype(f32),
        "wh1": wh1_h,
        "w2r": np.tile((head_w2.reshape(64) * invs4)[None, :],
                       (128, 1)).astype(BF),
    }

    # zero/ones plane: partitions 0/1 = 1.0, rest 0
    zp = np.zeros((128, TILE), f32)
    zp[0:2] = 1.0
    params["zpl"] = zp.astype(E4)

    # ---- per-core data tensors ----
    # d/m packed [B, 128, 2, N] fp8
    dmh = np.empty((B, C, 2, N), E4)
    dmh[:, :, 0, :] = np.clip(d_f * ad, 0, 240).transpose(0, 2, 1).astype(E4)
    dmh[:, :, 1, :] = np.clip(m_f * am, -240, 240).transpose(0, 2, 1).astype(E4)
    # but the weights were built for d_true = d_stored/ad with stored=d*ad:
    # inv1 built from w/ad so PSUM = k*(w.d_true*ad/ad)... handled above.

    # positions hi/mid/lo fp8, scaled by 1/4
    psc = np.concatenate([pos_A, pos_B], -1).transpose(0, 2, 1) * 0.25
    hi = np.clip(psc, -240, 240).astype(E4)
    r1 = psc - hi.astype(f32)
    mid = np.clip(r1, -240, 240).astype(E4)
    r2 = r1 - mid.astype(f32)
    lo = np.clip(r2, -240, 240).astype(E4)
    posq_h = np.concatenate([hi, mid, lo], axis=1)          # [B, 12, N]

    # q monomials (Hartley-normalized)
    srcn, sA, cxA, cyA = _hartley(pos_A)
    dstn, sB, cxB, cyB = _hartley(pos_B)
    sx, sy = srcn[..., 0], srcn[..., 1]
    dx, dy = dstn[..., 0], dstn[..., 1]
    one = np.ones_like(sx)
    q9 = np.stack([sx, sy, one, dx, dy, dx * sx, dx * sy, dy * sx, dy * sy],
                  axis=-1)                                   # [B, N, 9]

    in_maps = []
    for i in range(NCORES):
        sl = slice(i * BL, (i + 1) * BL)
        mcore = {"dm": np.ascontiguousarray(dmh[sl]),
                 "posq": np.ascontiguousarray(posq_h[sl])}
        mcore.update(params)
        in_maps.append(mcore)

    nc = _get_nc()
    res = bass_utils.run_bass_kernel_spmd(nc, in_maps,
                                          core_ids=list(range(NCORES)))
    wzr = np.concatenate([res.results[i]["out"] for i in range(NCORES)],
                         axis=0)                  # [B, NT, 128, NCH]
    wz = wzr.transpose(0, 1, 3, 2).reshape(B, N).astype(np.float64)
    w = 1.0 / (1.0 + np.exp(-(wz + float(head_b2[0]))))
    w = w.astype(f32)
    qw = q9 * w[:, :, None]
    M = np.einsum('bnk,bnl->bkl', qw, q9, optimize=True).astype(f32)

    # ---- host post: assemble AtWA/AtWb, solve, compose ----
    u3 = [0, 1, 2]
    AtWA = np.zeros((B, 8, 8), f32)
    AtWA[:, 0:3, 0:3] = M[:, 0:3, 0:3]
    AtWA[:, 3:6, 3:6] = M[:, 0:3, 0:3]
    AtWA[:, 0:3, 6] = -M[:, u3, 5]
    AtWA[:, 0:3, 7] = -M[:, u3, 6]
    AtWA[:, 3:6, 6] = -M[:, u3, 7]
    AtWA[:, 3:6, 7] = -M[:, u3, 8]
    AtWA[:, 6, 0:3] = -M[:, u3, 5]
    AtWA[:, 7, 0:3] = -M[:, u3, 6]
    AtWA[:, 6, 3:6] = -M[:, u3, 7]
    AtWA[:, 7, 3:6] = -M[:, u3, 8]
    AtWA[:, 6, 6] = M[:, 5, 5] + M[:, 7, 7]
    AtWA[:, 6, 7] = M[:, 5, 6] + M[:, 7, 8]
    AtWA[:, 7, 6] = M[:, 6, 5] + M[:, 8, 7]
    AtWA[:, 7, 7] = M[:, 6, 6] + M[:, 8, 8]
    AtWb = np.zeros((B, 8), f32)
    AtWb[:, 0:3] = M[:, 3, 0:3]
    AtWb[:, 3:6] = M[:, 4, 0:3]
    AtWb[:, 6] = -(M[:, 3, 5] + M[:, 4, 7])
    AtWb[:, 7] = -(M[:, 3, 6] + M[:, 4, 8])
    AtWA += REG * np.eye(8, dtype=f32)[None]
    h_id = np.array([1, 0, 0, 0, 1, 0, 0, 0], f32)
    AtWb += REG * h_id[None]

    try:
        h8 = np.linalg.solve(AtWA, AtWb[..., None])[..., 0].astype(f32)
    except np.linalg.LinAlgError:
        h8 = np.zeros((B, 8), f32)
        for b in range(B):
            try:
                h8[b] = np.linalg.solve(AtWA[b], AtWb[b])
            except np.linalg.LinAlgError:
                h8[b] = np.nan
    finite = np.all(np.isfinite(h8), axis=-1, keepdims=True)
    h8 = np.where(finite, h8, h_id[None])
    H_norm = np.concatenate([h8, np.ones((B, 1), f32)], axis=-1)
    H_norm = H_norm.reshape(B, 3, 3)

    T_src = np.zeros((B, 3, 3), f32)
    T_src[:, 0, 0] = sA
    T_src[:, 1, 1] = sA
    T_src[:, 0, 2] = -sA * cxA
    T_src[:, 1, 2] = -sA * cyA
    T_src[:, 2, 2] = 1.0
    s_dst = np.clip(sB, 1e-6, None)
    T_dst_inv = np.zeros((B, 3, 3), f32)
    T_dst_inv[:, 0, 0] = 1.0 / s_dst
    T_dst_inv[:, 1, 1] = 1.0 / s_dst
    T_dst_inv[:, 0, 2] = (sB * cxB) / s_dst
    T_dst_inv[:, 1, 2] = (sB * cyB) / s_dst
    T_dst_inv[:, 2, 2] = 1.0

    H = (T_dst_inv @ (H_norm @ T_src)).astype(f32)
    H = H / np.clip(np.abs(H[:, 2:3, 2:3]), 1e-8, None)
    h33 = H[:, 2:3, 2:3]
    sgn = np.sign(h33)
    sgn = np.where(sgn == 0, np.ones_like(sgn), sgn)
    H = H / (np.clip(np.abs(h33), 1e-8, None) * sgn)
    H_finite = np.all(np.isfinite(H), axis=(-2, -1))
    a33 = np.abs(H[:, 2, 2])
    valid = H_finite & (a33 > 1e-4) & (a33 < 1e4)
    eye = np.eye(3, dtype=f32)
    H = np.where(valid[:, None, None], H, eye[None])
    return H.astype(f32)
